# revision 3
# baseline (speedup 1.0000x reference)
"""Dual-RoPE attention block (B=8, S=1024, 16 heads x 64) on 8 NeuronCores.

Single fused pipeline around the exp stream on ScalarE (the ~1.1us/tile
metronome, 256 [128,1024] tiles per core): every other engine pipelines
around it and the PE never idles long enough for the HAM clock gate to
re-throttle.  PSUM budget (8 banks): 2x scores tile [128,1024]f32 +
2x PV tile [65,1024]f32; projection chunks borrow scores buffers right
after a pass's first two score tiles are in flight.

Sharding: data-parallel over batch, one batch element per core.

Per-core dataflow (all matmuls bf16 inputs, fp32 PSUM accumulation):
  - head-pair 0's q/k projected from a tiny pre-sliced weight tensor
    (wqk0) so the first scores don't wait for the full 4MB wqkT load;
    input DMAs spread across the SP/Activation/GpSimd queues.
  - 4 V-projection chunks run in the DMA-bound ramp, 4 interleave with
    head-pair 0's first pass; q/k chunks of pair cc+1 are emitted after
    pass-1's first scores of pair cc so their RoPE (VectorE) overlaps
    pass 1 and the scores rotation keeps ScalarE fed.
  - scores row-tiled 2x (K=64; heads at partitions 0-63 / 64-127 run
    concurrently in the PE array).
  - PV with the 65-row trick: vext column blocks are [2.0-const | v], so PV
    row 0 yields 2*sum_k(exp); rec = recip(2 sum) folds pass-averaging.
  - softmax without max-subtraction (scores O(10), exp safe in fp32).
  - normalize split in two phases so the PV psum frees early; rec
    broadcast via gpsimd.partition_broadcast; output projection pipelined
    (per-sc partial over heads 0-6 while the last normalize completes).
"""

import numpy as np
import ml_dtypes

B, S, DM = 8, 1024, 1024
NH, HD = 16, 64
HD1 = HD + 1
NC = 8                # cores

# Schraudolph-on-DVE key-chunk assignment (kc values whose exp runs on
# VectorE instead of ScalarE). () disables.
DVE_KCS = ()
SCH_A = 184.6630
SCH_B = 16249.5

_CACHE = {}


def _build(dve_kcs=DVE_KCS):
    key = ("final", tuple(dve_kcs))
    if key in _CACHE:
        return _CACHE[key]
    from concourse import bacc, mybir
    import concourse.tile as tile

    f32 = mybir.dt.float32
    bf16 = mybir.dt.bfloat16
    i16 = mybir.dt.int16
    EXP = mybir.ActivationFunctionType.Exp
    MULT = mybir.AluOpType.mult
    ADD = mybir.AluOpType.add

    nc = bacc.Bacc("TRN2", target_bir_lowering=False, debug=False,
                   enable_asserts=False, num_devices=NC)

    xT_d = nc.dram_tensor("xT", [DM, S], bf16, kind="ExternalInput").ap()
    wqkT_d = nc.dram_tensor("wqkT", [DM, 2 * DM], bf16, kind="ExternalInput").ap()
    wqk0_d = nc.dram_tensor("wqk0", [128, 2048], bf16, kind="ExternalInput").ap()
    wvT_d = nc.dram_tensor("wvT", [DM, DM], bf16, kind="ExternalInput").ap()
    woT_d = nc.dram_tensor("woT", [DM, DM], bf16, kind="ExternalInput").ap()
    trig_d = nc.dram_tensor("trig", [128, 4 * S], bf16, kind="ExternalInput").ap()
    out_d = nc.dram_tensor("out", [S, DM], f32, kind="ExternalOutput").ap()

    with tile.TileContext(nc) as tc:
        with (
            tc.tile_pool(name="persist", bufs=1) as pp,
            tc.tile_pool(name="qkt", bufs=3) as qp,
            tc.tile_pool(name="expp", bufs=6) as ep,
            tc.tile_pool(name="smal", bufs=3) as sp,
            tc.tile_pool(name="bigps", bufs=2, space="PSUM") as bps,
            tc.tile_pool(name="pvps", bufs=1, space="PSUM") as pvp_pool,
        ):
            # ---------- persistent tiles + input DMA (ordered by need) -----
            xT_sb = [pp.tile([128, S], bf16, name=f"xT{i}") for i in range(8)]
            wqkT_sb = [pp.tile([128, 2 * DM], bf16, name=f"wqk{i}")
                       for i in range(8)]
            wvT_sb = [pp.tile([128, DM], bf16, name=f"wv{i}") for i in range(8)]
            woT_sb = [pp.tile([128, DM], bf16, name=f"woT{i}") for i in range(8)]
            trig_sb = pp.tile([128, 4 * S], bf16, name="trig")
            trigC_t = [trig_sb[:, p * S:(p + 1) * S] for p in range(2)]
            trigS_t = [trig_sb[:, (2 + p) * S:(3 + p) * S] for p in range(2)]
            vext = [pp.tile([128, NH * HD1], bf16, name=f"vext{i}")
                    for i in range(8)]
            attn_b = [pp.tile([128, S], bf16, name=f"attnb{i}") for i in range(8)]

            wqk0_sb = pp.tile([128, 2048], bf16, name="wqk0")
            nc.sync.dma_start(wqk0_sb[:], wqk0_d[:])
            nc.scalar.dma_start(trig_sb[:], trig_d[:])
            for i in range(8):
                eng = nc.sync if i % 2 == 0 else nc.scalar
                eng.dma_start(xT_sb[i][:], xT_d[i * 128:(i + 1) * 128, :])
            for i in range(8):
                nc.sync.dma_start(wqkT_sb[i][:],
                                  wqkT_d[i * 128:(i + 1) * 128, :])
            for i in range(8):
                # off the SP queue so the rotate-half swap strips (critical
                # path of the first rope) aren't stuck behind bulk weights
                nc.gpsimd.dma_start(wvT_sb[i][:], wvT_d[i * 128:(i + 1) * 128, :])
            for i in range(8):
                nc.gpsimd.dma_start(woT_sb[i][:], woT_d[i * 128:(i + 1) * 128, :])

            # roped q/k for both passes, double-buffered across cc:
            # roped[cc%2][pss][0]=q chunk, [1]=k chunk
            roped = [[[pp.tile([128, S], bf16, name=f"rope{par}_{p}_{t}")
                       for t in range(2)] for p in range(2)] for par in range(2)]

            def qk_proj_chunk(cc, which):
                """Project chunk `which` (0=q, 1=k) of head-pair cc into
                [c, s] layout and RoPE it for both passes."""
                wcol = cc + 8 * which
                ps = bps.tile([128, S], f32, tag="big", bufs=2,
                              name=f"qkps{cc}_{which}")
                for dc in range(8):
                    if cc == 0:
                        wsl = wqk0_sb[:, dc * 256 + which * 128:
                                      dc * 256 + (which + 1) * 128]
                    else:
                        wsl = wqkT_sb[dc][:, wcol * 128:(wcol + 1) * 128]
                    for n in range(2):
                        nc.tensor.matmul(
                            ps[:, n * 512:(n + 1) * 512],
                            wsl,
                            xT_sb[dc][:, n * 512:(n + 1) * 512],
                            start=(dc == 0), stop=(dc == 7))
                qk = qp.tile([128, S], bf16, tag="qk", bufs=2,
                             name=f"qk{cc}_{which}")
                nc.vector.tensor_copy(qk[:], ps[:])
                # partition-swapped (rotate_half) copy via SBUF DMA strips
                sw = qp.tile([128, S], bf16, tag="sw", bufs=2,
                             name=f"sw{cc}_{which}")
                for hh in range(2):
                    for f in range(2):
                        o0 = hh * 64 + f * 32
                        i0 = hh * 64 + (1 - f) * 32
                        nc.sync.dma_start(sw[o0:o0 + 32, :],
                                          qk[i0:i0 + 32, :])
                for pss in range(2):
                    a = qp.tile([128, S], bf16, tag="ropeA", bufs=1,
                                name=f"ropeA{cc}_{which}_{pss}")
                    bb = qp.tile([128, S], bf16, tag="ropeB", bufs=1,
                                 name=f"ropeB{cc}_{which}_{pss}")
                    nc.vector.tensor_mul(a[:], qk[:], trigC_t[pss][:])
                    nc.vector.tensor_mul(bb[:], sw[:], trigS_t[pss][:])
                    nc.vector.tensor_add(roped[cc % 2][pss][which][:],
                                         a[:], bb[:])

            def v_proj(sc):
                """Project V chunk sc (128 seq rows) into vext[sc] with the
                2.0-const column 0 per head."""
                ps = bps.tile([128, S], f32, tag="big", bufs=2,
                              name=f"vps{sc}")
                for dc in range(8):
                    for n in range(2):
                        nc.tensor.matmul(
                            ps[:, n * 512:(n + 1) * 512],
                            xT_sb[dc][:, sc * 128:(sc + 1) * 128],
                            wvT_sb[dc][:, n * 512:(n + 1) * 512],
                            start=(dc == 0), stop=(dc == 7))
                vv = vext[sc][:].rearrange("p (h e) -> p h e", e=HD1)
                nc.vector.tensor_copy(
                    vv[:, :, 1:HD1],
                    ps[:].rearrange("p (h e) -> p h e", e=HD))
                nc.vector.memset(vv[:, :, 0:1], 2.0)

            def attention(cc, pss, extras=None):
                """Both heads (2cc, 2cc+1) of pass pss.  extras: dict
                kc -> thunk emitted after that kc's exp (PE filler work)."""
                hE, hO = 2 * cc, 2 * cc + 1
                q1 = roped[cc % 2][pss][0]
                k1 = roped[cc % 2][pss][1]
                pvps = [pvp_pool.tile([HD1, S], f32, tag="pvps", bufs=2,
                                      name=f"pvp{pss}_{2 * cc + g}")
                        for g in range(2)]
                for kc in range(8):
                    es_t = []
                    for n in range(2):
                        scp = bps.tile([128, S], f32, tag="big", bufs=2,
                                       name=f"scp{pss}_{cc}_{kc}_{n}")
                        for g, hh in ((0, 0), (1, 64)):
                            nc.tensor.matmul(
                                scp[:, g * 512:(g + 1) * 512],
                                k1[hh:hh + 64, kc * 128:(kc + 1) * 128],
                                q1[hh:hh + 64, n * 512:(n + 1) * 512],
                                start=True, stop=True)
                        es = ep.tile([128, S], bf16, tag="expS", bufs=5,
                                     name=f"es{pss}_{cc}_{kc}_{n}")
                        if kc in dve_kcs:
                            nc.vector.tensor_scalar(
                                es[:].bitcast(i16), scp[:],
                                0.125 * SCH_A, SCH_B, MULT, ADD)
                        else:
                            nc.scalar.activation(es[:], scp[:], EXP,
                                                 scale=0.125)
                        es_t.append(es)
                    if extras and kc in extras:
                        extras[kc]()
                    # PV for this kc (stationary reused across n)
                    for g, h in ((0, hE), (1, hO)):
                        for n in range(2):
                            nc.tensor.matmul(
                                pvps[g][:, n * 512:(n + 1) * 512],
                                vext[kc][:, h * HD1:(h + 1) * HD1],
                                es_t[n][:, g * 512:(g + 1) * 512],
                                start=(kc == 0), stop=(kc == 7))

                # phase A of the normalize: free the PV psum tiles ASAP
                ab = []
                for g, h in ((0, hE), (1, hO)):
                    pvp = pvps[g]
                    pv_sb = sp.tile([HD1, S], bf16, tag="pvsb", bufs=3,
                                    name=f"pvsb{pss}_{h}")
                    nc.vector.tensor_copy(pv_sb[:], pvp[:])
                    recf = sp.tile([1, S], f32, tag="recf", bufs=2,
                                   name=f"recf{pss}_{h}")
                    nc.vector.reciprocal_approx_fast(recf[0:1, :],
                                                     pvp[0:1, :])
                    ab.append((pv_sb, recf))
                return ab

            def norm_phase_b(ab, cc, pss):
                hE, hO = 2 * cc, 2 * cc + 1
                cts = []
                for g, h in ((0, hE), (1, hO)):
                    pv_sb, recf = ab[g]
                    rec = sp.tile([1, S], bf16, tag="rec", bufs=2,
                                  name=f"rec{pss}_{h}")
                    with nc.allow_low_precision(
                            reason="bf16 recip of softmax sums"):
                        nc.vector.tensor_copy(rec[0:1, :], recf[0:1, :])
                    bc_sb = sp.tile([HD1, S], bf16, tag="bcsb", bufs=1,
                                    name=f"bcsb{pss}_{h}")
                    nc.gpsimd.partition_broadcast(bc_sb[:, :], rec[0:1, :],
                                                  channels=HD1)
                    ct = sp.tile([HD1, S], bf16, tag=f"ct{pss}{g}", bufs=1,
                                 name=f"ct{pss}_{h}")
                    nc.vector.tensor_mul(ct[:], pv_sb[:], bc_sb[:])
                    cts.append(ct)
                return cts

            # ---------- fused pipeline ----------
            qk_proj_chunk(0, 0)
            qk_proj_chunk(0, 1)
            for s in range(4):
                v_proj(s)

            for cc in range(8):
                ex0 = {kc: (lambda s=kc + 4: v_proj(s))
                       for kc in range(4)} if cc == 0 else None
                ab0 = attention(cc, 0, extras=ex0)
                pair_cts = norm_phase_b(ab0, cc, 0)
                ex1 = ({0: (lambda c=cc: qk_proj_chunk(c + 1, 0)),
                        1: (lambda c=cc: qk_proj_chunk(c + 1, 1))}
                       if cc < 7 else None)
                ab1 = attention(cc, 1, extras=ex1)
                pair_cts += norm_phase_b(ab1, cc, 1)
                for g, h in ((0, 2 * cc), (1, 2 * cc + 1)):
                    hh = (h % 2) * 64
                    ah = sp.tile([HD1, S], bf16, tag="ah", bufs=2,
                                 name=f"ah{h}")
                    nc.vector.tensor_add(ah[:], pair_cts[g][:],
                                         pair_cts[2 + g][:])
                    nc.sync.dma_start(attn_b[cc][hh:hh + 64, :], ah[1:HD1, :])

            # ---------- output projection ----------
            # partial accumulation over heads 0-6 of the next sc chunk is
            # emitted before the cc=7 finisher of the current one, so the PE
            # works while the last head-pair's normalize chain completes.
            def oproj_partial(sc, op):
                for cc in range(7):
                    for n in range(2):
                        nc.tensor.matmul(
                            op[:, n * 512:(n + 1) * 512],
                            attn_b[cc][:, sc * 128:(sc + 1) * 128],
                            woT_sb[cc][:, n * 512:(n + 1) * 512],
                            start=(cc == 0), stop=False)

            def oproj_finish(sc, op):
                for n in range(2):
                    nc.tensor.matmul(
                        op[:, n * 512:(n + 1) * 512],
                        attn_b[7][:, sc * 128:(sc + 1) * 128],
                        woT_sb[7][:, n * 512:(n + 1) * 512],
                        start=False, stop=True)
                ob = sp.tile([128, DM], f32, tag="ob", bufs=2,
                             name=f"ob{sc}")
                nc.vector.tensor_copy(ob[:], op[:])
                eng = nc.sync if sc % 2 == 0 else nc.scalar
                eng.dma_start(out_d[sc * 128:(sc + 1) * 128, :], ob[:])

            ops = {}
            ops[0] = bps.tile([128, DM], f32, tag="big", bufs=2, name="op0")
            oproj_partial(0, ops[0])
            for sc in range(8):
                if sc + 1 < 8:
                    ops[sc + 1] = bps.tile([128, DM], f32, tag="big",
                                           bufs=2, name=f"op{sc + 1}")
                    oproj_partial(sc + 1, ops[sc + 1])
                oproj_finish(sc, ops.pop(sc))

    nc.compile()
    _CACHE[key] = nc
    return nc


def _prep_inputs(hidden_states, cos, sin, w_qkv, w_o):
    bf = ml_dtypes.bfloat16
    xT = np.ascontiguousarray(
        hidden_states.transpose(0, 2, 1)).astype(bf)          # [B, DM, S]
    wqkT = np.ascontiguousarray(w_qkv[:2 * DM].T).astype(bf)  # [DM, 2DM]
    wvT = np.ascontiguousarray(w_qkv[2 * DM:].T).astype(bf)   # [DM, DM]
    woT = np.ascontiguousarray(w_o.T).astype(bf)              # [DM, DM]

    idx = np.arange(S).reshape(32, 32).T.reshape(-1)
    d = np.arange(128) % HD
    sign = np.where(d < 32, -1.0, 1.0).astype(np.float32)
    trig = np.concatenate([
        cos[:, d].T, cos[idx][:, d].T,
        sin[:, d].T * sign[:, None], sin[idx][:, d].T * sign[:, None],
    ], axis=1).astype(bf)                                     # [128, 4S]
    wqk0 = np.ascontiguousarray(
        np.concatenate([wqkT[:, 0:128], wqkT[:, 1024:1152]], axis=1)
        .reshape(8, 128, 256).transpose(1, 0, 2).reshape(128, 2048))
    shared = {"wqkT": wqkT, "wqk0": wqk0, "wvT": wvT, "woT": woT,
              "trig": np.ascontiguousarray(trig)}
    return [{"xT": np.ascontiguousarray(xT[b]), **shared} for b in range(B)]


def _install_ntff_hook():
    import sys, types
    if "antenv.axon_hooks" in sys.modules:
        return
    try:
        from trn_agent_boot.trn_boot import _ntff_profile_via_ctypes
        hook = _ntff_profile_via_ctypes('/opt/axon/libaxon_pjrt.so')
    except Exception:
        hook = None
    mod = types.ModuleType("antenv.axon_hooks")
    mod.get_axon_ntff_profile_hook = lambda: hook
    mod.set_axon_ntff_profile_hook = lambda h: None
    sys.modules["antenv.axon_hooks"] = mod


def kernel(hidden_states, cos, sin, w_qkv, w_o, _trace=False, _tmpdir=None):
    from concourse import bass_utils
    if _trace:
        _install_ntff_hook()
    nc = _build()
    in_maps = _prep_inputs(np.asarray(hidden_states, np.float32),
                           np.asarray(cos, np.float32),
                           np.asarray(sin, np.float32),
                           np.asarray(w_qkv, np.float32),
                           np.asarray(w_o, np.float32))
    res = bass_utils.run_bass_kernel_spmd(
        nc, in_maps, core_ids=list(range(NC)),
        trace=_trace, tmpdir=_tmpdir)
    out = np.stack([np.asarray(res.results[b]["out"], np.float32)
                    for b in range(B)])
    kernel.last_exec_time_ns = res.exec_time_ns
    return out


# revision 4
# speedup vs baseline: 1.0961x; 1.0961x over previous
"""Dual-RoPE attention block (B=8, S=1024, 16 heads x 64) on 8 NeuronCores.

v2: single fused pipeline so ScalarE (exp) starts ~130us earlier and all
engines stay busy end-to-end.  PSUM budget (8 banks): 2x scores tile
[128,1024]f32 (4 banks) + 2x PV tile [65,1024]f32 (4 banks); projection
chunks borrow the scores buffers in pass-tail windows.

Sharding: data-parallel over batch, one batch element per core.

Per-core dataflow (all matmuls bf16 inputs, fp32 PSUM accumulation):
  - qk-proj of head-pair 0 upfront; V-projection chunks interleaved with
    head-pair 0's first attention pass (chunk kc emitted just before the
    PV that consumes it); thereafter q-chunk of cc+1 projected in the tail
    of pass 0, k-chunk in the tail of pass 1.
  - scores row-tiled 2x (K=64; heads at partitions 0-63 / 64-127 run
    concurrently in the PE array).
  - exp on ScalarE; optionally a subset of key-chunks on VectorE via a
    bf16 Schraudolph bit-trick (tensor_scalar -> int16 -> bitcast bf16).
  - PV with the 65-row trick: vext column blocks are [2.0-const | v], so PV
    row 0 yields 2*sum_k(exp); rec = recip(2 sum) folds pass-averaging.
  - softmax without max-subtraction (scores O(10), exp safe in fp32).
  - rec broadcast via gpsimd.partition_broadcast; normalize on DVE in bf16.
"""

import numpy as np
import ml_dtypes

B, S, DM = 8, 1024, 1024
NH, HD = 16, 64
HD1 = HD + 1
NC = 8                # cores

# Schraudolph-on-DVE key-chunk assignment (kc values whose exp runs on
# VectorE instead of ScalarE). () disables.
DVE_KCS = ()
SCH_A = 184.6630
SCH_B = 16249.5

_CACHE = {}


def _build(dve_kcs=DVE_KCS):
    key = ("final", tuple(dve_kcs))
    if key in _CACHE:
        return _CACHE[key]
    from concourse import bacc, mybir
    import concourse.tile as tile

    f32 = mybir.dt.float32
    bf16 = mybir.dt.bfloat16
    i16 = mybir.dt.int16
    EXP = mybir.ActivationFunctionType.Exp
    MULT = mybir.AluOpType.mult
    ADD = mybir.AluOpType.add

    nc = bacc.Bacc("TRN2", target_bir_lowering=False, debug=False,
                   enable_asserts=False, num_devices=NC)

    xT_d = nc.dram_tensor("xT", [DM, S], bf16, kind="ExternalInput").ap()
    wqkT_d = nc.dram_tensor("wqkT", [DM, 2 * DM], bf16, kind="ExternalInput").ap()
    wqk0_d = nc.dram_tensor("wqk0", [128, 2048], bf16, kind="ExternalInput").ap()
    wvT_d = nc.dram_tensor("wvT", [DM, DM], bf16, kind="ExternalInput").ap()
    woT_d = nc.dram_tensor("woT", [DM, DM], bf16, kind="ExternalInput").ap()
    trig_d = nc.dram_tensor("trig", [128, 4 * S], bf16, kind="ExternalInput").ap()
    out_d = nc.dram_tensor("out", [S, DM], f32, kind="ExternalOutput").ap()

    with tile.TileContext(nc) as tc:
        with (
            tc.tile_pool(name="persist", bufs=1) as pp,
            tc.tile_pool(name="qkt", bufs=3) as qp,
            tc.tile_pool(name="expp", bufs=6) as ep,
            tc.tile_pool(name="smal", bufs=3) as sp,
            tc.tile_pool(name="bigps", bufs=2, space="PSUM") as bps,
            tc.tile_pool(name="pvps", bufs=1, space="PSUM") as pvp_pool,
        ):
            # ---------- persistent tiles + input DMA (ordered by need) -----
            xT_sb = [pp.tile([128, S], bf16, name=f"xT{i}") for i in range(8)]
            wqkT_sb = [pp.tile([128, 2 * DM], bf16, name=f"wqk{i}")
                       for i in range(8)]
            wvT_sb = [pp.tile([128, DM], bf16, name=f"wv{i}") for i in range(8)]
            woT_sb = [pp.tile([128, DM], bf16, name=f"woT{i}") for i in range(8)]
            trig_sb = pp.tile([128, 4 * S], bf16, name="trig")
            trigC_t = [trig_sb[:, p * S:(p + 1) * S] for p in range(2)]
            trigS_t = [trig_sb[:, (2 + p) * S:(3 + p) * S] for p in range(2)]
            vext = [pp.tile([128, NH * HD1], bf16, name=f"vext{i}")
                    for i in range(8)]
            attn_b = [pp.tile([128, S], bf16, name=f"attnb{i}") for i in range(8)]

            wqk0_sb = pp.tile([128, 2048], bf16, name="wqk0")
            # xT gates the first projection: split it across both HWDGE
            # queues ahead of everything except the tiny wqk0 slice.
            nc.sync.dma_start(wqk0_sb[:], wqk0_d[:])
            for i in range(8):
                eng = nc.sync if i % 2 == 0 else nc.scalar
                eng.dma_start(xT_sb[i][:], xT_d[i * 128:(i + 1) * 128, :])
            nc.scalar.dma_start(trig_sb[:], trig_d[:])
            # bulk weights stay off the SP queue: the rotate-half swap strips
            # (critical path of every rope) are issued there on demand
            for i in range(8):
                nc.gpsimd.dma_start(wvT_sb[i][:], wvT_d[i * 128:(i + 1) * 128, :])
            for i in range(8):
                nc.scalar.dma_start(wqkT_sb[i][:],
                                    wqkT_d[i * 128:(i + 1) * 128, :])
            for i in range(8):
                nc.gpsimd.dma_start(woT_sb[i][:], woT_d[i * 128:(i + 1) * 128, :])

            # roped q/k for both passes, double-buffered across cc:
            # roped[cc%2][pss][0]=q chunk, [1]=k chunk
            roped = [[[pp.tile([128, S], bf16, name=f"rope{par}_{p}_{t}")
                       for t in range(2)] for p in range(2)] for par in range(2)]

            def qk_proj_chunk(cc, which):
                """Project chunk `which` (0=q, 1=k) of head-pair cc into
                [c, s] layout and RoPE it for both passes."""
                wcol = cc + 8 * which
                ps = bps.tile([128, S], f32, tag="big", bufs=2,
                              name=f"qkps{cc}_{which}")
                for dc in range(8):
                    if cc == 0:
                        wsl = wqk0_sb[:, dc * 256 + which * 128:
                                      dc * 256 + (which + 1) * 128]
                    else:
                        wsl = wqkT_sb[dc][:, wcol * 128:(wcol + 1) * 128]
                    for n in range(2):
                        nc.tensor.matmul(
                            ps[:, n * 512:(n + 1) * 512],
                            wsl,
                            xT_sb[dc][:, n * 512:(n + 1) * 512],
                            start=(dc == 0), stop=(dc == 7))
                qk = qp.tile([128, S], bf16, tag="qk", bufs=2,
                             name=f"qk{cc}_{which}")
                nc.vector.tensor_copy(qk[:], ps[:])
                # partition-swapped (rotate_half) copy via SBUF DMA strips
                sw = qp.tile([128, S], bf16, tag="sw", bufs=2,
                             name=f"sw{cc}_{which}")
                for hh in range(2):
                    for f in range(2):
                        o0 = hh * 64 + f * 32
                        i0 = hh * 64 + (1 - f) * 32
                        nc.sync.dma_start(sw[o0:o0 + 32, :],
                                          qk[i0:i0 + 32, :])
                for pss in range(2):
                    a = qp.tile([128, S], bf16, tag="ropeA", bufs=1,
                                name=f"ropeA{cc}_{which}_{pss}")
                    bb = qp.tile([128, S], bf16, tag="ropeB", bufs=1,
                                 name=f"ropeB{cc}_{which}_{pss}")
                    nc.vector.tensor_mul(a[:], qk[:], trigC_t[pss][:])
                    nc.vector.tensor_mul(bb[:], sw[:], trigS_t[pss][:])
                    nc.vector.tensor_add(roped[cc % 2][pss][which][:],
                                         a[:], bb[:])

            def v_proj(sc):
                """Project V chunk sc (128 seq rows) into vext[sc] with the
                2.0-const column 0 per head."""
                ps = bps.tile([128, S], f32, tag="big", bufs=2,
                              name=f"vps{sc}")
                for dc in range(8):
                    for n in range(2):
                        nc.tensor.matmul(
                            ps[:, n * 512:(n + 1) * 512],
                            xT_sb[dc][:, sc * 128:(sc + 1) * 128],
                            wvT_sb[dc][:, n * 512:(n + 1) * 512],
                            start=(dc == 0), stop=(dc == 7))
                vv = vext[sc][:].rearrange("p (h e) -> p h e", e=HD1)
                nc.vector.tensor_copy(
                    vv[:, :, 1:HD1],
                    ps[:].rearrange("p (h e) -> p h e", e=HD))
                nc.vector.memset(vv[:, :, 0:1], 2.0)

            def attention(cc, pss, extras=None):
                """Both heads (2cc, 2cc+1) of pass pss.  extras: dict
                kc -> thunk emitted after that kc's exp (PE filler work)."""
                hE, hO = 2 * cc, 2 * cc + 1
                q1 = roped[cc % 2][pss][0]
                k1 = roped[cc % 2][pss][1]
                pvps = [pvp_pool.tile([HD1, S], f32, tag="pvps", bufs=2,
                                      name=f"pvp{pss}_{2 * cc + g}")
                        for g in range(2)]
                for kc in range(8):
                    es_t = []
                    for n in range(2):
                        scp = bps.tile([128, S], f32, tag="big", bufs=2,
                                       name=f"scp{pss}_{cc}_{kc}_{n}")
                        for g, hh in ((0, 0), (1, 64)):
                            nc.tensor.matmul(
                                scp[:, g * 512:(g + 1) * 512],
                                k1[hh:hh + 64, kc * 128:(kc + 1) * 128],
                                q1[hh:hh + 64, n * 512:(n + 1) * 512],
                                start=True, stop=True)
                        es = ep.tile([128, S], bf16, tag="expS", bufs=5,
                                     name=f"es{pss}_{cc}_{kc}_{n}")
                        if kc in dve_kcs:
                            nc.vector.tensor_scalar(
                                es[:].bitcast(i16), scp[:],
                                0.125 * SCH_A, SCH_B, MULT, ADD)
                        else:
                            nc.scalar.activation(es[:], scp[:], EXP,
                                                 scale=0.125)
                        es_t.append(es)
                    if extras and kc in extras:
                        extras[kc]()
                    # PV for this kc (stationary reused across n)
                    for g, h in ((0, hE), (1, hO)):
                        for n in range(2):
                            nc.tensor.matmul(
                                pvps[g][:, n * 512:(n + 1) * 512],
                                vext[kc][:, h * HD1:(h + 1) * HD1],
                                es_t[n][:, g * 512:(g + 1) * 512],
                                start=(kc == 0), stop=(kc == 7))

                # phase A of the normalize: free the PV psum tiles ASAP
                ab = []
                for g, h in ((0, hE), (1, hO)):
                    pvp = pvps[g]
                    pv_sb = sp.tile([HD1, S], bf16, tag="pvsb", bufs=3,
                                    name=f"pvsb{pss}_{h}")
                    nc.vector.tensor_copy(pv_sb[:], pvp[:])
                    recf = sp.tile([1, S], f32, tag="recf", bufs=2,
                                   name=f"recf{pss}_{h}")
                    nc.vector.reciprocal_approx_fast(recf[0:1, :],
                                                     pvp[0:1, :])
                    ab.append((pv_sb, recf))
                return ab

            def norm_phase_b(ab, cc, pss):
                hE, hO = 2 * cc, 2 * cc + 1
                cts = []
                for g, h in ((0, hE), (1, hO)):
                    pv_sb, recf = ab[g]
                    rec = sp.tile([1, S], bf16, tag="rec", bufs=2,
                                  name=f"rec{pss}_{h}")
                    with nc.allow_low_precision(
                            reason="bf16 recip of softmax sums"):
                        nc.vector.tensor_copy(rec[0:1, :], recf[0:1, :])
                    bc_sb = sp.tile([HD1, S], bf16, tag="bcsb", bufs=1,
                                    name=f"bcsb{pss}_{h}")
                    nc.gpsimd.partition_broadcast(bc_sb[:, :], rec[0:1, :],
                                                  channels=HD1)
                    ct = sp.tile([HD1, S], bf16, tag=f"ct{pss}{g}", bufs=1,
                                 name=f"ct{pss}_{h}")
                    nc.vector.tensor_mul(ct[:], pv_sb[:], bc_sb[:])
                    cts.append(ct)
                return cts

            # ---------- fused pipeline ----------
            qk_proj_chunk(0, 0)
            qk_proj_chunk(0, 1)
            for s in range(4):
                v_proj(s)

            for cc in range(8):
                ex0 = {kc: (lambda s=kc + 4: v_proj(s))
                       for kc in range(4)} if cc == 0 else None
                ab0 = attention(cc, 0, extras=ex0)
                pair_cts = norm_phase_b(ab0, cc, 0)
                ex1 = ({0: (lambda c=cc: qk_proj_chunk(c + 1, 0)),
                        1: (lambda c=cc: qk_proj_chunk(c + 1, 1))}
                       if cc < 7 else None)
                ab1 = attention(cc, 1, extras=ex1)
                pair_cts += norm_phase_b(ab1, cc, 1)
                for g, h in ((0, 2 * cc), (1, 2 * cc + 1)):
                    hh = (h % 2) * 64
                    ah = sp.tile([HD1, S], bf16, tag="ah", bufs=2,
                                 name=f"ah{h}")
                    nc.vector.tensor_add(ah[:], pair_cts[g][:],
                                         pair_cts[2 + g][:])
                    nc.sync.dma_start(attn_b[cc][hh:hh + 64, :], ah[1:HD1, :])

            # ---------- output projection ----------
            # partial accumulation over heads 0-6 of the next sc chunk is
            # emitted before the cc=7 finisher of the current one, so the PE
            # works while the last head-pair's normalize chain completes.
            def oproj_partial(sc, op):
                for cc in range(7):
                    for n in range(2):
                        nc.tensor.matmul(
                            op[:, n * 512:(n + 1) * 512],
                            attn_b[cc][:, sc * 128:(sc + 1) * 128],
                            woT_sb[cc][:, n * 512:(n + 1) * 512],
                            start=(cc == 0), stop=False)

            def oproj_finish(sc, op):
                for n in range(2):
                    nc.tensor.matmul(
                        op[:, n * 512:(n + 1) * 512],
                        attn_b[7][:, sc * 128:(sc + 1) * 128],
                        woT_sb[7][:, n * 512:(n + 1) * 512],
                        start=False, stop=True)
                ob = sp.tile([128, DM], f32, tag="ob", bufs=2,
                             name=f"ob{sc}")
                nc.vector.tensor_copy(ob[:], op[:])
                eng = nc.sync if sc % 2 == 0 else nc.scalar
                eng.dma_start(out_d[sc * 128:(sc + 1) * 128, :], ob[:])

            ops = {}
            ops[0] = bps.tile([128, DM], f32, tag="big", bufs=2, name="op0")
            oproj_partial(0, ops[0])
            for sc in range(8):
                if sc + 1 < 8:
                    ops[sc + 1] = bps.tile([128, DM], f32, tag="big",
                                           bufs=2, name=f"op{sc + 1}")
                    oproj_partial(sc + 1, ops[sc + 1])
                oproj_finish(sc, ops.pop(sc))

    nc.compile()
    _CACHE[key] = nc
    return nc


def _prep_inputs(hidden_states, cos, sin, w_qkv, w_o):
    bf = ml_dtypes.bfloat16
    xT = np.ascontiguousarray(
        hidden_states.transpose(0, 2, 1)).astype(bf)          # [B, DM, S]
    wqkT = np.ascontiguousarray(w_qkv[:2 * DM].T).astype(bf)  # [DM, 2DM]
    wvT = np.ascontiguousarray(w_qkv[2 * DM:].T).astype(bf)   # [DM, DM]
    woT = np.ascontiguousarray(w_o.T).astype(bf)              # [DM, DM]

    idx = np.arange(S).reshape(32, 32).T.reshape(-1)
    d = np.arange(128) % HD
    sign = np.where(d < 32, -1.0, 1.0).astype(np.float32)
    trig = np.concatenate([
        cos[:, d].T, cos[idx][:, d].T,
        sin[:, d].T * sign[:, None], sin[idx][:, d].T * sign[:, None],
    ], axis=1).astype(bf)                                     # [128, 4S]
    wqk0 = np.ascontiguousarray(
        np.concatenate([wqkT[:, 0:128], wqkT[:, 1024:1152]], axis=1)
        .reshape(8, 128, 256).transpose(1, 0, 2).reshape(128, 2048))
    shared = {"wqkT": wqkT, "wqk0": wqk0, "wvT": wvT, "woT": woT,
              "trig": np.ascontiguousarray(trig)}
    return [{"xT": np.ascontiguousarray(xT[b]), **shared} for b in range(B)]


def _install_ntff_hook():
    import sys, types
    if "antenv.axon_hooks" in sys.modules:
        return
    try:
        from trn_agent_boot.trn_boot import _ntff_profile_via_ctypes
        hook = _ntff_profile_via_ctypes('/opt/axon/libaxon_pjrt.so')
    except Exception:
        hook = None
    mod = types.ModuleType("antenv.axon_hooks")
    mod.get_axon_ntff_profile_hook = lambda: hook
    mod.set_axon_ntff_profile_hook = lambda h: None
    sys.modules["antenv.axon_hooks"] = mod


def kernel(hidden_states, cos, sin, w_qkv, w_o, _trace=False, _tmpdir=None):
    from concourse import bass_utils
    if _trace:
        _install_ntff_hook()
    nc = _build()
    in_maps = _prep_inputs(np.asarray(hidden_states, np.float32),
                           np.asarray(cos, np.float32),
                           np.asarray(sin, np.float32),
                           np.asarray(w_qkv, np.float32),
                           np.asarray(w_o, np.float32))
    res = bass_utils.run_bass_kernel_spmd(
        nc, in_maps, core_ids=list(range(NC)),
        trace=_trace, tmpdir=_tmpdir)
    out = np.stack([np.asarray(res.results[b]["out"], np.float32)
                    for b in range(B)])
    kernel.last_exec_time_ns = res.exec_time_ns
    return out


# revision 5
# speedup vs baseline: 1.1182x; 1.0202x over previous
"""Dual-RoPE attention block (B=8, S=1024, 16 heads x 64) on 8 NeuronCores.

v2: single fused pipeline so ScalarE (exp) starts ~130us earlier and all
engines stay busy end-to-end.  PSUM budget (8 banks): 2x scores tile
[128,1024]f32 (4 banks) + 2x PV tile [65,1024]f32 (4 banks); projection
chunks borrow the scores buffers in pass-tail windows.

Sharding: data-parallel over batch, one batch element per core.

Per-core dataflow (all matmuls bf16 inputs, fp32 PSUM accumulation):
  - qk-proj of head-pair 0 upfront; V-projection chunks interleaved with
    head-pair 0's first attention pass (chunk kc emitted just before the
    PV that consumes it); thereafter q-chunk of cc+1 projected in the tail
    of pass 0, k-chunk in the tail of pass 1.
  - scores row-tiled 2x (K=64; heads at partitions 0-63 / 64-127 run
    concurrently in the PE array).
  - exp on ScalarE; optionally a subset of key-chunks on VectorE via a
    bf16 Schraudolph bit-trick (tensor_scalar -> int16 -> bitcast bf16).
  - PV with the 65-row trick: vext column blocks are [2.0-const | v], so PV
    row 0 yields 2*sum_k(exp); rec = recip(2 sum) folds pass-averaging.
  - softmax without max-subtraction (scores O(10), exp safe in fp32).
  - rec broadcast via gpsimd.partition_broadcast; normalize on DVE in bf16.
"""

import numpy as np
import ml_dtypes

B, S, DM = 8, 1024, 1024
NH, HD = 16, 64
HD1 = HD + 1
NC = 8                # cores

# Schraudolph-on-DVE key-chunk assignment (kc values whose exp runs on
# VectorE instead of ScalarE). () disables.
DVE_KCS = ()
SCH_A = 184.6630
SCH_B = 16249.5

_CACHE = {}


def _build(dve_kcs=DVE_KCS):
    key = ("final", tuple(dve_kcs))
    if key in _CACHE:
        return _CACHE[key]
    from concourse import bacc, mybir
    import concourse.tile as tile

    f32 = mybir.dt.float32
    bf16 = mybir.dt.bfloat16
    i16 = mybir.dt.int16
    EXP = mybir.ActivationFunctionType.Exp
    MULT = mybir.AluOpType.mult
    ADD = mybir.AluOpType.add

    nc = bacc.Bacc("TRN2", target_bir_lowering=False, debug=False,
                   enable_asserts=False, num_devices=NC)

    xT_d = nc.dram_tensor("xT", [DM, S], bf16, kind="ExternalInput").ap()
    wqkT_d = nc.dram_tensor("wqkT", [DM, 2 * DM], bf16, kind="ExternalInput").ap()
    wqk0_d = nc.dram_tensor("wqk0", [128, 2048], bf16, kind="ExternalInput").ap()
    wvT_d = nc.dram_tensor("wvT", [DM, DM], bf16, kind="ExternalInput").ap()
    woT_d = nc.dram_tensor("woT", [DM, DM], bf16, kind="ExternalInput").ap()
    trig_d = nc.dram_tensor("trig", [128, 4 * S], bf16, kind="ExternalInput").ap()
    out_d = nc.dram_tensor("out", [S, DM], f32, kind="ExternalOutput").ap()

    with tile.TileContext(nc) as tc:
        with (
            tc.tile_pool(name="persist", bufs=1) as pp,
            tc.tile_pool(name="qkt", bufs=3) as qp,
            tc.tile_pool(name="expp", bufs=6) as ep,
            tc.tile_pool(name="smal", bufs=3) as sp,
            tc.tile_pool(name="bigps", bufs=2, space="PSUM") as bps,
            tc.tile_pool(name="pvps", bufs=1, space="PSUM") as pvp_pool,
        ):
            # ---------- persistent tiles + input DMA (ordered by need) -----
            xT_sb = [pp.tile([128, S], bf16, name=f"xT{i}") for i in range(8)]
            wqkT_sb = [pp.tile([128, 2 * DM], bf16, name=f"wqk{i}")
                       for i in range(8)]
            wvT_sb = [pp.tile([128, DM], bf16, name=f"wv{i}") for i in range(8)]
            woT_sb = [pp.tile([128, DM], bf16, name=f"woT{i}") for i in range(8)]
            trig_sb = pp.tile([128, 4 * S], bf16, name="trig")
            trigC_t = [trig_sb[:, p * S:(p + 1) * S] for p in range(2)]
            trigS_t = [trig_sb[:, (2 + p) * S:(3 + p) * S] for p in range(2)]
            vext = [pp.tile([128, NH * HD1], bf16, name=f"vext{i}")
                    for i in range(8)]
            attn_b = [pp.tile([128, S], bf16, name=f"attnb{i}") for i in range(8)]

            wqk0_sb = pp.tile([128, 2048], bf16, name="wqk0")
            # xT gates the first projection: split it across both HWDGE
            # queues ahead of everything except the tiny wqk0 slice.
            nc.sync.dma_start(wqk0_sb[:], wqk0_d[:])
            for i in range(8):
                eng = nc.sync if i % 2 == 0 else nc.scalar
                eng.dma_start(xT_sb[i][:], xT_d[i * 128:(i + 1) * 128, :])
            nc.scalar.dma_start(trig_sb[:], trig_d[:])
            # bulk weights stay off the SP queue: the rotate-half swap strips
            # (critical path of every rope) are issued there on demand
            for i in range(8):
                nc.gpsimd.dma_start(wvT_sb[i][:], wvT_d[i * 128:(i + 1) * 128, :])
            for i in range(8):
                nc.scalar.dma_start(wqkT_sb[i][:],
                                    wqkT_d[i * 128:(i + 1) * 128, :])
            for i in range(8):
                nc.gpsimd.dma_start(woT_sb[i][:], woT_d[i * 128:(i + 1) * 128, :])

            # roped q/k for both passes, double-buffered across cc:
            # roped[cc%2][pss][0]=q chunk, [1]=k chunk
            roped = [[[pp.tile([128, S], bf16, name=f"rope{par}_{p}_{t}")
                       for t in range(2)] for p in range(2)] for par in range(2)]

            def qk_proj_chunk(cc, which):
                """Project chunk `which` (0=q, 1=k) of head-pair cc into
                [c, s] layout and RoPE it for both passes."""
                wcol = cc + 8 * which
                ps = bps.tile([128, S], f32, tag="big", bufs=3,
                              name=f"qkps{cc}_{which}")
                for dc in range(8):
                    if cc == 0:
                        wsl = wqk0_sb[:, dc * 256 + which * 128:
                                      dc * 256 + (which + 1) * 128]
                    else:
                        wsl = wqkT_sb[dc][:, wcol * 128:(wcol + 1) * 128]
                    for n in range(2):
                        nc.tensor.matmul(
                            ps[:, n * 512:(n + 1) * 512],
                            wsl,
                            xT_sb[dc][:, n * 512:(n + 1) * 512],
                            start=(dc == 0), stop=(dc == 7))
                qk = qp.tile([128, S], bf16, tag="qk", bufs=2,
                             name=f"qk{cc}_{which}")
                nc.vector.tensor_copy(qk[:], ps[:])
                # partition-swapped (rotate_half) copy via SBUF DMA strips
                sw = qp.tile([128, S], bf16, tag="sw", bufs=2,
                             name=f"sw{cc}_{which}")
                for hh in range(2):
                    for f in range(2):
                        o0 = hh * 64 + f * 32
                        i0 = hh * 64 + (1 - f) * 32
                        nc.sync.dma_start(sw[o0:o0 + 32, :],
                                          qk[i0:i0 + 32, :])
                for pss in range(2):
                    a = qp.tile([128, S], bf16, tag="ropeA", bufs=1,
                                name=f"ropeA{cc}_{which}_{pss}")
                    bb = qp.tile([128, S], bf16, tag="ropeB", bufs=1,
                                 name=f"ropeB{cc}_{which}_{pss}")
                    nc.vector.tensor_mul(a[:], qk[:], trigC_t[pss][:])
                    nc.vector.tensor_mul(bb[:], sw[:], trigS_t[pss][:])
                    nc.vector.tensor_add(roped[cc % 2][pss][which][:],
                                         a[:], bb[:])

            def v_proj(sc):
                """Project V chunk sc (128 seq rows) into vext[sc] with the
                2.0-const column 0 per head."""
                ps = bps.tile([128, S], f32, tag="big", bufs=3,
                              name=f"vps{sc}")
                for dc in range(8):
                    for n in range(2):
                        nc.tensor.matmul(
                            ps[:, n * 512:(n + 1) * 512],
                            xT_sb[dc][:, sc * 128:(sc + 1) * 128],
                            wvT_sb[dc][:, n * 512:(n + 1) * 512],
                            start=(dc == 0), stop=(dc == 7))
                vv = vext[sc][:].rearrange("p (h e) -> p h e", e=HD1)
                nc.vector.tensor_copy(
                    vv[:, :, 1:HD1],
                    ps[:].rearrange("p (h e) -> p h e", e=HD))
                nc.vector.memset(vv[:, :, 0:1], 2.0)

            # Phase = (cc, pss, n-half): scores+exp of a phase run while
            # the PREVIOUS phase's PV accumulates underneath (per-kc
            # interleave).  PV psum is then 2x [65,512] (2 banks) instead
            # of 4, buying a THIRD scores buffer: projection insertions
            # hold one buf while two still rotate, so the exp stream on
            # ScalarE never starves.
            quarters = {}
            nquart = {}

            def finish_phase(pd, pvt):
                cc, pss, n, _ = pd
                hE, hO = 2 * cc, 2 * cc + 1
                for g, h in ((0, hE), (1, hO)):
                    pvp = pvt[g]
                    pv_sb = sp.tile([HD1, 512], bf16, tag="pvsb", bufs=4,
                                    name=f"pvsb{pss}_{h}_{n}")
                    nc.vector.tensor_copy(pv_sb[:], pvp[:])
                    recf = sp.tile([1, 512], f32, tag="recf", bufs=2,
                                   name=f"recf{pss}_{h}_{n}")
                    nc.vector.reciprocal_approx_fast(recf[0:1, :],
                                                     pvp[0:1, :])
                    rec = sp.tile([1, 512], bf16, tag="rec", bufs=2,
                                  name=f"rec{pss}_{h}_{n}")
                    with nc.allow_low_precision(
                            reason="bf16 recip of softmax sums"):
                        nc.vector.tensor_copy(rec[0:1, :], recf[0:1, :])
                    bc_sb = sp.tile([HD1, 512], bf16, tag="bcsb", bufs=2,
                                    name=f"bcsb{pss}_{h}_{n}")
                    nc.gpsimd.partition_broadcast(bc_sb[:, :], rec[0:1, :],
                                                  channels=HD1)
                    ct = sp.tile([HD1, 512], bf16, tag=f"ct{pss}{g}{n}",
                                 bufs=1, name=f"ct{pss}_{h}_{n}")
                    nc.vector.tensor_mul(ct[:], pv_sb[:], bc_sb[:])
                    quarters[(cc, pss, g, n)] = ct
                nquart[cc] = nquart.get(cc, 0) + 2
                if nquart[cc] == 8:
                    for g, h in ((0, hE), (1, hO)):
                        hh = (h % 2) * 64
                        ah = sp.tile([HD1, S], bf16, tag="ah", bufs=2,
                                     name=f"ah{h}")
                        for nn in range(2):
                            nc.vector.tensor_add(
                                ah[:, nn * 512:(nn + 1) * 512],
                                quarters.pop((cc, 0, g, nn))[:],
                                quarters.pop((cc, 1, g, nn))[:])
                        nc.sync.dma_start(attn_b[cc][hh:hh + 64, :],
                                          ah[1:HD1, :])

            def run_phase(cc, pss, n, pending, extras=None):
                q1 = roped[cc % 2][pss][0]
                k1 = roped[cc % 2][pss][1]
                pvt = None
                if pending is not None:
                    pcc = pending[0]
                    pvt = [pvp_pool.tile([HD1, 512], f32, tag="pv", bufs=2,
                                         name=f"pv{pcc}_{g}")
                           for g in range(2)]
                es_list = []
                for kc in range(8):
                    scp = bps.tile([128, S], f32, tag="big", bufs=3,
                                   name=f"scp{pss}_{cc}_{kc}_{n}")
                    for g, hh in ((0, 0), (1, 64)):
                        nc.tensor.matmul(
                            scp[:, g * 512:(g + 1) * 512],
                            k1[hh:hh + 64, kc * 128:(kc + 1) * 128],
                            q1[hh:hh + 64, n * 512:(n + 1) * 512],
                            start=True, stop=True)
                    es = ep.tile([128, S], bf16, tag="expS", bufs=12,
                                 name=f"es{pss}_{cc}_{kc}_{n}")
                    nc.scalar.activation(es[:], scp[:], EXP, scale=0.125)
                    if extras and kc in extras:
                        extras[kc]()
                    if pending is not None:
                        pcc, ppss, pn, pes = pending
                        for g, h in ((0, 2 * pcc), (1, 2 * pcc + 1)):
                            nc.tensor.matmul(
                                pvt[g][:, :],
                                vext[kc][:, h * HD1:(h + 1) * HD1],
                                pes[kc][:, g * 512:(g + 1) * 512],
                                start=(kc == 0), stop=(kc == 7))
                    es_list.append(es)
                if pending is not None:
                    finish_phase(pending, pvt)
                return (cc, pss, n, es_list)

            # ---------- fused pipeline ----------
            qk_proj_chunk(0, 0)
            qk_proj_chunk(0, 1)
            v_proj(0)
            v_proj(1)

            pending = None
            for cc in range(8):
                for pss in range(2):
                    for n in range(2):
                        ex = None
                        if cc == 0 and pss == 0:
                            ex = {kc: (lambda s=3 * n + kc: v_proj(s))
                                  for kc in range(3 if n == 0 else 3)}
                            ex = {kc: (lambda s=2 + 3 * n + kc: v_proj(s))
                                  for kc in range(3)}
                        elif cc < 7 and pss == 1 and n == 0:
                            ex = {0: (lambda c=cc: qk_proj_chunk(c + 1, 0)),
                                  1: (lambda c=cc: qk_proj_chunk(c + 1, 1))}
                        pending = run_phase(cc, pss, n, pending, extras=ex)
            # drain the last phase's PV
            pcc = pending[0]
            pvt = [pvp_pool.tile([HD1, 512], f32, tag="pv", bufs=2,
                                 name=f"pvD_{g}") for g in range(2)]
            for kc in range(8):
                pes = pending[3]
                for g, h in ((0, 2 * pcc), (1, 2 * pcc + 1)):
                    nc.tensor.matmul(
                        pvt[g][:, :], vext[kc][:, h * HD1:(h + 1) * HD1],
                        pes[kc][:, g * 512:(g + 1) * 512],
                        start=(kc == 0), stop=(kc == 7))
            finish_phase(pending, pvt)

            # ---------- output projection ----------
            # partial accumulation over heads 0-6 of the next sc chunk is
            # emitted before the cc=7 finisher of the current one, so the PE
            # works while the last head-pair's normalize chain completes.
            def oproj_partial(sc, op):
                for cc in range(7):
                    for n in range(2):
                        nc.tensor.matmul(
                            op[:, n * 512:(n + 1) * 512],
                            attn_b[cc][:, sc * 128:(sc + 1) * 128],
                            woT_sb[cc][:, n * 512:(n + 1) * 512],
                            start=(cc == 0), stop=False)

            def oproj_finish(sc, op):
                for n in range(2):
                    nc.tensor.matmul(
                        op[:, n * 512:(n + 1) * 512],
                        attn_b[7][:, sc * 128:(sc + 1) * 128],
                        woT_sb[7][:, n * 512:(n + 1) * 512],
                        start=False, stop=True)
                ob = sp.tile([128, DM], f32, tag="ob", bufs=1,
                             name=f"ob{sc}")
                nc.vector.tensor_copy(ob[:], op[:])
                eng = nc.sync if sc % 2 == 0 else nc.scalar
                eng.dma_start(out_d[sc * 128:(sc + 1) * 128, :], ob[:])

            ops = {}
            ops[0] = bps.tile([128, DM], f32, tag="big", bufs=3, name="op0")
            oproj_partial(0, ops[0])
            for sc in range(8):
                if sc + 1 < 8:
                    ops[sc + 1] = bps.tile([128, DM], f32, tag="big",
                                           bufs=3, name=f"op{sc + 1}")
                    oproj_partial(sc + 1, ops[sc + 1])
                oproj_finish(sc, ops.pop(sc))

    nc.compile()
    _CACHE[key] = nc
    return nc


def _prep_inputs(hidden_states, cos, sin, w_qkv, w_o):
    bf = ml_dtypes.bfloat16
    xT = np.ascontiguousarray(
        hidden_states.transpose(0, 2, 1)).astype(bf)          # [B, DM, S]
    wqkT = np.ascontiguousarray(w_qkv[:2 * DM].T).astype(bf)  # [DM, 2DM]
    wvT = np.ascontiguousarray(w_qkv[2 * DM:].T).astype(bf)   # [DM, DM]
    woT = np.ascontiguousarray(w_o.T).astype(bf)              # [DM, DM]

    idx = np.arange(S).reshape(32, 32).T.reshape(-1)
    d = np.arange(128) % HD
    sign = np.where(d < 32, -1.0, 1.0).astype(np.float32)
    trig = np.concatenate([
        cos[:, d].T, cos[idx][:, d].T,
        sin[:, d].T * sign[:, None], sin[idx][:, d].T * sign[:, None],
    ], axis=1).astype(bf)                                     # [128, 4S]
    wqk0 = np.ascontiguousarray(
        np.concatenate([wqkT[:, 0:128], wqkT[:, 1024:1152]], axis=1)
        .reshape(8, 128, 256).transpose(1, 0, 2).reshape(128, 2048))
    shared = {"wqkT": wqkT, "wqk0": wqk0, "wvT": wvT, "woT": woT,
              "trig": np.ascontiguousarray(trig)}
    return [{"xT": np.ascontiguousarray(xT[b]), **shared} for b in range(B)]


def _install_ntff_hook():
    import sys, types
    if "antenv.axon_hooks" in sys.modules:
        return
    try:
        from trn_agent_boot.trn_boot import _ntff_profile_via_ctypes
        hook = _ntff_profile_via_ctypes('/opt/axon/libaxon_pjrt.so')
    except Exception:
        hook = None
    mod = types.ModuleType("antenv.axon_hooks")
    mod.get_axon_ntff_profile_hook = lambda: hook
    mod.set_axon_ntff_profile_hook = lambda h: None
    sys.modules["antenv.axon_hooks"] = mod


def kernel(hidden_states, cos, sin, w_qkv, w_o, _trace=False, _tmpdir=None):
    from concourse import bass_utils
    if _trace:
        _install_ntff_hook()
    nc = _build()
    in_maps = _prep_inputs(np.asarray(hidden_states, np.float32),
                           np.asarray(cos, np.float32),
                           np.asarray(sin, np.float32),
                           np.asarray(w_qkv, np.float32),
                           np.asarray(w_o, np.float32))
    res = bass_utils.run_bass_kernel_spmd(
        nc, in_maps, core_ids=list(range(NC)),
        trace=_trace, tmpdir=_tmpdir)
    out = np.stack([np.asarray(res.results[b]["out"], np.float32)
                    for b in range(B)])
    kernel.last_exec_time_ns = res.exec_time_ns
    return out


# revision 6
# speedup vs baseline: 1.1579x; 1.0355x over previous
"""Dual-RoPE attention block (B=8, S=1024, 16 heads x 64) on 8 NeuronCores.

Single fused pipeline around the exp stream on ScalarE (the ~1.1us/tile
metronome, 256 [128,1024] tiles per core).  PSUM budget (8 banks):
3x scores tile [128,1024]f32 (6 banks) + 2x PV tile [65,512]f32 (2
banks): PV accumulation is sequenced by (pass, query-half) phase and
runs underneath the NEXT phase's scores, so projection insertions can
hold one scores buffer while two still rotate and ScalarE never
starves.

Sharding: data-parallel over batch, one batch element per core.

Per-core dataflow (all matmuls bf16 inputs, fp32 PSUM accumulation):
  - qk-proj of head-pair 0 upfront; V-projection chunks interleaved with
    head-pair 0's first attention pass (chunk kc emitted just before the
    PV that consumes it); thereafter q-chunk of cc+1 projected in the tail
    of pass 0, k-chunk in the tail of pass 1.
  - scores row-tiled 2x (K=64; heads at partitions 0-63 / 64-127 run
    concurrently in the PE array).
  - exp on ScalarE; optionally a subset of key-chunks on VectorE via a
    bf16 Schraudolph bit-trick (tensor_scalar -> int16 -> bitcast bf16).
  - PV with the 65-row trick: vext column blocks are [2.0-const | v], so PV
    row 0 yields 2*sum_k(exp); rec = recip(2 sum) folds pass-averaging.
  - softmax without max-subtraction (scores O(10), exp safe in fp32).
  - rec broadcast via gpsimd.partition_broadcast; normalize on DVE in bf16.
"""

import numpy as np
import ml_dtypes

B, S, DM = 8, 1024, 1024
NH, HD = 16, 64
HD1 = HD + 1
NC = 8                # cores

# Schraudolph-on-DVE key-chunk assignment (kc values whose exp runs on
# VectorE instead of ScalarE). () disables.
DVE_KCS = ()
SCH_A = 184.6630
SCH_B = 16249.5

_CACHE = {}


def _build(dve_kcs=DVE_KCS):
    key = ("final", tuple(dve_kcs))
    if key in _CACHE:
        return _CACHE[key]
    from concourse import bacc, mybir
    import concourse.tile as tile

    f32 = mybir.dt.float32
    bf16 = mybir.dt.bfloat16
    i16 = mybir.dt.int16
    EXP = mybir.ActivationFunctionType.Exp
    MULT = mybir.AluOpType.mult
    ADD = mybir.AluOpType.add

    nc = bacc.Bacc("TRN2", target_bir_lowering=False, debug=False,
                   enable_asserts=False, num_devices=NC)

    xT_d = nc.dram_tensor("xT", [DM, S], bf16, kind="ExternalInput").ap()
    wqkT_d = nc.dram_tensor("wqkT", [DM, 2 * DM], bf16, kind="ExternalInput").ap()
    wqk0_d = nc.dram_tensor("wqk0", [128, 2048], bf16, kind="ExternalInput").ap()
    wvT_d = nc.dram_tensor("wvT", [DM, DM], bf16, kind="ExternalInput").ap()
    woT_d = nc.dram_tensor("woT", [DM, DM], bf16, kind="ExternalInput").ap()
    trig_d = nc.dram_tensor("trig", [128, 4 * S], bf16, kind="ExternalInput").ap()
    out_d = nc.dram_tensor("out", [S, DM], f32, kind="ExternalOutput").ap()

    with tile.TileContext(nc) as tc:
        with (
            tc.tile_pool(name="persist", bufs=1) as pp,
            tc.tile_pool(name="qkt", bufs=3) as qp,
            tc.tile_pool(name="expp", bufs=6) as ep,
            tc.tile_pool(name="smal", bufs=3) as sp,
            tc.tile_pool(name="bigps", bufs=2, space="PSUM") as bps,
            tc.tile_pool(name="pvps", bufs=1, space="PSUM") as pvp_pool,
        ):
            # ---------- persistent tiles + input DMA (ordered by need) -----
            xT_sb = [pp.tile([128, S], bf16, name=f"xT{i}") for i in range(8)]
            wqkT_sb = [pp.tile([128, 2 * DM], bf16, name=f"wqk{i}")
                       for i in range(8)]
            wvT_sb = [pp.tile([128, DM], bf16, name=f"wv{i}") for i in range(8)]
            woT_sb = [pp.tile([128, DM], bf16, name=f"woT{i}") for i in range(8)]
            trig_sb = pp.tile([128, 4 * S], bf16, name="trig")
            trigC_t = [trig_sb[:, p * S:(p + 1) * S] for p in range(2)]
            trigS_t = [trig_sb[:, (2 + p) * S:(3 + p) * S] for p in range(2)]
            vext = [pp.tile([128, NH * HD1], bf16, name=f"vext{i}")
                    for i in range(8)]
            attn_b = [pp.tile([128, S], bf16, name=f"attnb{i}") for i in range(8)]

            wqk0_sb = pp.tile([128, 2048], bf16, name="wqk0")
            # xT gates the first projection: split it across both HWDGE
            # queues ahead of everything except the tiny wqk0 slice.
            nc.sync.dma_start(wqk0_sb[:], wqk0_d[:])
            for i in range(8):
                eng = nc.sync if i % 2 == 0 else nc.scalar
                eng.dma_start(xT_sb[i][:], xT_d[i * 128:(i + 1) * 128, :])
            nc.scalar.dma_start(trig_sb[:], trig_d[:])
            # bulk weights stay off the SP queue: the rotate-half swap strips
            # (critical path of every rope) are issued there on demand
            for i in range(8):
                nc.gpsimd.dma_start(wvT_sb[i][:], wvT_d[i * 128:(i + 1) * 128, :])
            for i in range(8):
                nc.scalar.dma_start(wqkT_sb[i][:],
                                    wqkT_d[i * 128:(i + 1) * 128, :])
            for i in range(8):
                nc.gpsimd.dma_start(woT_sb[i][:], woT_d[i * 128:(i + 1) * 128, :])

            # roped q/k for both passes, double-buffered across cc:
            # roped[cc%2][pss][0]=q chunk, [1]=k chunk
            roped = [[[pp.tile([128, S], bf16, name=f"rope{par}_{p}_{t}")
                       for t in range(2)] for p in range(2)] for par in range(2)]

            def qk_proj_chunk(cc, which):
                """Project chunk `which` (0=q, 1=k) of head-pair cc into
                [c, s] layout and RoPE it for both passes."""
                wcol = cc + 8 * which
                ps = bps.tile([128, S], f32, tag="big", bufs=3,
                              name=f"qkps{cc}_{which}")
                for dc in range(8):
                    if cc == 0:
                        wsl = wqk0_sb[:, dc * 256 + which * 128:
                                      dc * 256 + (which + 1) * 128]
                    else:
                        wsl = wqkT_sb[dc][:, wcol * 128:(wcol + 1) * 128]
                    for n in range(2):
                        nc.tensor.matmul(
                            ps[:, n * 512:(n + 1) * 512],
                            wsl,
                            xT_sb[dc][:, n * 512:(n + 1) * 512],
                            start=(dc == 0), stop=(dc == 7))
                qk = qp.tile([128, S], bf16, tag="qk", bufs=2,
                             name=f"qk{cc}_{which}")
                nc.vector.tensor_copy(qk[:], ps[:])
                # partition-swapped (rotate_half) copy via SBUF DMA strips
                sw = qp.tile([128, S], bf16, tag="sw", bufs=2,
                             name=f"sw{cc}_{which}")
                for hh in range(2):
                    for f in range(2):
                        o0 = hh * 64 + f * 32
                        i0 = hh * 64 + (1 - f) * 32
                        nc.sync.dma_start(sw[o0:o0 + 32, :],
                                          qk[i0:i0 + 32, :])
                for pss in range(2):
                    a = qp.tile([128, S], bf16, tag="ropeA", bufs=1,
                                name=f"ropeA{cc}_{which}_{pss}")
                    bb = qp.tile([128, S], bf16, tag="ropeB", bufs=1,
                                 name=f"ropeB{cc}_{which}_{pss}")
                    nc.vector.tensor_mul(a[:], qk[:], trigC_t[pss][:])
                    nc.vector.tensor_mul(bb[:], sw[:], trigS_t[pss][:])
                    nc.vector.tensor_add(roped[cc % 2][pss][which][:],
                                         a[:], bb[:])

            def v_proj(sc):
                """Project V chunk sc (128 seq rows) into vext[sc] with the
                2.0-const column 0 per head."""
                ps = bps.tile([128, S], f32, tag="big", bufs=3,
                              name=f"vps{sc}")
                for dc in range(8):
                    for n in range(2):
                        nc.tensor.matmul(
                            ps[:, n * 512:(n + 1) * 512],
                            xT_sb[dc][:, sc * 128:(sc + 1) * 128],
                            wvT_sb[dc][:, n * 512:(n + 1) * 512],
                            start=(dc == 0), stop=(dc == 7))
                vv = vext[sc][:].rearrange("p (h e) -> p h e", e=HD1)
                nc.vector.tensor_copy(
                    vv[:, :, 1:HD1],
                    ps[:].rearrange("p (h e) -> p h e", e=HD))
                nc.vector.memset(vv[:, :, 0:1], 2.0)

            # Phase = (cc, pss, n-half): scores+exp of a phase run while
            # the PREVIOUS phase's PV accumulates underneath (per-kc
            # interleave).  PV psum is then 2x [65,512] (2 banks) instead
            # of 4, buying a THIRD scores buffer: projection insertions
            # hold one buf while two still rotate, so the exp stream on
            # ScalarE never starves.
            quarters = {}
            nquart = {}

            def finish_phase(pd, pvt):
                cc, pss, n, _ = pd
                hE, hO = 2 * cc, 2 * cc + 1
                for g, h in ((0, hE), (1, hO)):
                    pvp = pvt[g]
                    pv_sb = sp.tile([HD1, 512], bf16, tag="pvsb", bufs=4,
                                    name=f"pvsb{pss}_{h}_{n}")
                    nc.vector.tensor_copy(pv_sb[:], pvp[:])
                    recf = sp.tile([1, 512], f32, tag="recf", bufs=2,
                                   name=f"recf{pss}_{h}_{n}")
                    nc.vector.reciprocal_approx_fast(recf[0:1, :],
                                                     pvp[0:1, :])
                    rec = sp.tile([1, 512], bf16, tag="rec", bufs=2,
                                  name=f"rec{pss}_{h}_{n}")
                    with nc.allow_low_precision(
                            reason="bf16 recip of softmax sums"):
                        nc.vector.tensor_copy(rec[0:1, :], recf[0:1, :])
                    bc_sb = sp.tile([HD1, 512], bf16, tag="bcsb", bufs=2,
                                    name=f"bcsb{pss}_{h}_{n}")
                    nc.gpsimd.partition_broadcast(bc_sb[:, :], rec[0:1, :],
                                                  channels=HD1)
                    ct = sp.tile([HD1, 512], bf16, tag=f"ct{pss}{g}{n}",
                                 bufs=1, name=f"ct{pss}_{h}_{n}")
                    nc.vector.tensor_mul(ct[:], pv_sb[:], bc_sb[:])
                    quarters[(cc, pss, g, n)] = ct
                nquart[cc] = nquart.get(cc, 0) + 2
                if nquart[cc] == 8:
                    for g, h in ((0, hE), (1, hO)):
                        hh = (h % 2) * 64
                        ah = sp.tile([HD1, S], bf16, tag="ah", bufs=2,
                                     name=f"ah{h}")
                        for nn in range(2):
                            nc.vector.tensor_add(
                                ah[:, nn * 512:(nn + 1) * 512],
                                quarters.pop((cc, 0, g, nn))[:],
                                quarters.pop((cc, 1, g, nn))[:])
                        nc.sync.dma_start(attn_b[cc][hh:hh + 64, :],
                                          ah[1:HD1, :])

            def run_phase(cc, pss, n, pending, extras=None):
                q1 = roped[cc % 2][pss][0]
                k1 = roped[cc % 2][pss][1]
                pvt = None
                if pending is not None:
                    pcc = pending[0]
                    pvt = [pvp_pool.tile([HD1, 512], f32, tag="pv", bufs=2,
                                         name=f"pv{pcc}_{g}")
                           for g in range(2)]
                es_list = []
                for kc in range(8):
                    scp = bps.tile([128, S], f32, tag="big", bufs=3,
                                   name=f"scp{pss}_{cc}_{kc}_{n}")
                    for g, hh in ((0, 0), (1, 64)):
                        nc.tensor.matmul(
                            scp[:, g * 512:(g + 1) * 512],
                            k1[hh:hh + 64, kc * 128:(kc + 1) * 128],
                            q1[hh:hh + 64, n * 512:(n + 1) * 512],
                            start=True, stop=True)
                    es = ep.tile([128, S], bf16, tag="expS", bufs=12,
                                 name=f"es{pss}_{cc}_{kc}_{n}")
                    nc.scalar.activation(es[:], scp[:], EXP, scale=0.125)
                    if extras and kc in extras:
                        extras[kc]()
                    if pending is not None:
                        pcc, ppss, pn, pes = pending
                        for g, h in ((0, 2 * pcc), (1, 2 * pcc + 1)):
                            nc.tensor.matmul(
                                pvt[g][:, :],
                                vext[kc][:, h * HD1:(h + 1) * HD1],
                                pes[kc][:, g * 512:(g + 1) * 512],
                                start=(kc == 0), stop=(kc == 7))
                    es_list.append(es)
                if pending is not None:
                    finish_phase(pending, pvt)
                return (cc, pss, n, es_list)

            # ---------- fused pipeline ----------
            qk_proj_chunk(0, 0)
            qk_proj_chunk(0, 1)
            v_proj(0)
            v_proj(1)

            pending = None
            for cc in range(8):
                for pss in range(2):
                    for n in range(2):
                        ex = None
                        if cc == 0 and pss == 0:
                            ex = {kc: (lambda s=3 * n + kc: v_proj(s))
                                  for kc in range(3 if n == 0 else 3)}
                            ex = {kc: (lambda s=2 + 3 * n + kc: v_proj(s))
                                  for kc in range(3)}
                        elif cc < 7 and pss == 1 and n == 0:
                            ex = {0: (lambda c=cc: qk_proj_chunk(c + 1, 0)),
                                  4: (lambda c=cc: qk_proj_chunk(c + 1, 1))}
                        pending = run_phase(cc, pss, n, pending, extras=ex)
            # drain the last phase's PV
            pcc = pending[0]
            pvt = [pvp_pool.tile([HD1, 512], f32, tag="pv", bufs=2,
                                 name=f"pvD_{g}") for g in range(2)]
            for kc in range(8):
                pes = pending[3]
                for g, h in ((0, 2 * pcc), (1, 2 * pcc + 1)):
                    nc.tensor.matmul(
                        pvt[g][:, :], vext[kc][:, h * HD1:(h + 1) * HD1],
                        pes[kc][:, g * 512:(g + 1) * 512],
                        start=(kc == 0), stop=(kc == 7))
            finish_phase(pending, pvt)

            # ---------- output projection ----------
            # partial accumulation over heads 0-6 of the next sc chunk is
            # emitted before the cc=7 finisher of the current one, so the PE
            # works while the last head-pair's normalize chain completes.
            def oproj_partial(sc, op):
                for cc in range(7):
                    for n in range(2):
                        nc.tensor.matmul(
                            op[:, n * 512:(n + 1) * 512],
                            attn_b[cc][:, sc * 128:(sc + 1) * 128],
                            woT_sb[cc][:, n * 512:(n + 1) * 512],
                            start=(cc == 0), stop=False)

            def oproj_finish(sc, op):
                for n in range(2):
                    nc.tensor.matmul(
                        op[:, n * 512:(n + 1) * 512],
                        attn_b[7][:, sc * 128:(sc + 1) * 128],
                        woT_sb[7][:, n * 512:(n + 1) * 512],
                        start=False, stop=True)
                ob = sp.tile([128, DM], f32, tag="ob", bufs=1,
                             name=f"ob{sc}")
                nc.vector.tensor_copy(ob[:], op[:])
                eng = nc.sync if sc % 2 == 0 else nc.scalar
                eng.dma_start(out_d[sc * 128:(sc + 1) * 128, :], ob[:])

            ops = {}
            ops[0] = bps.tile([128, DM], f32, tag="big", bufs=3, name="op0")
            oproj_partial(0, ops[0])
            for sc in range(8):
                if sc + 1 < 8:
                    ops[sc + 1] = bps.tile([128, DM], f32, tag="big",
                                           bufs=3, name=f"op{sc + 1}")
                    oproj_partial(sc + 1, ops[sc + 1])
                oproj_finish(sc, ops.pop(sc))

    nc.compile()
    _CACHE[key] = nc
    return nc


def _prep_inputs(hidden_states, cos, sin, w_qkv, w_o):
    bf = ml_dtypes.bfloat16
    xT = np.ascontiguousarray(
        hidden_states.transpose(0, 2, 1)).astype(bf)          # [B, DM, S]
    wqkT = np.ascontiguousarray(w_qkv[:2 * DM].T).astype(bf)  # [DM, 2DM]
    wvT = np.ascontiguousarray(w_qkv[2 * DM:].T).astype(bf)   # [DM, DM]
    woT = np.ascontiguousarray(w_o.T).astype(bf)              # [DM, DM]

    idx = np.arange(S).reshape(32, 32).T.reshape(-1)
    d = np.arange(128) % HD
    sign = np.where(d < 32, -1.0, 1.0).astype(np.float32)
    trig = np.concatenate([
        cos[:, d].T, cos[idx][:, d].T,
        sin[:, d].T * sign[:, None], sin[idx][:, d].T * sign[:, None],
    ], axis=1).astype(bf)                                     # [128, 4S]
    wqk0 = np.ascontiguousarray(
        np.concatenate([wqkT[:, 0:128], wqkT[:, 1024:1152]], axis=1)
        .reshape(8, 128, 256).transpose(1, 0, 2).reshape(128, 2048))
    shared = {"wqkT": wqkT, "wqk0": wqk0, "wvT": wvT, "woT": woT,
              "trig": np.ascontiguousarray(trig)}
    return [{"xT": np.ascontiguousarray(xT[b]), **shared} for b in range(B)]


def _install_ntff_hook():
    import sys, types
    if "antenv.axon_hooks" in sys.modules:
        return
    try:
        from trn_agent_boot.trn_boot import _ntff_profile_via_ctypes
        hook = _ntff_profile_via_ctypes('/opt/axon/libaxon_pjrt.so')
    except Exception:
        hook = None
    mod = types.ModuleType("antenv.axon_hooks")
    mod.get_axon_ntff_profile_hook = lambda: hook
    mod.set_axon_ntff_profile_hook = lambda h: None
    sys.modules["antenv.axon_hooks"] = mod


def kernel(hidden_states, cos, sin, w_qkv, w_o, _trace=False, _tmpdir=None):
    from concourse import bass_utils
    if _trace:
        _install_ntff_hook()
    nc = _build()
    in_maps = _prep_inputs(np.asarray(hidden_states, np.float32),
                           np.asarray(cos, np.float32),
                           np.asarray(sin, np.float32),
                           np.asarray(w_qkv, np.float32),
                           np.asarray(w_o, np.float32))
    res = bass_utils.run_bass_kernel_spmd(
        nc, in_maps, core_ids=list(range(NC)),
        trace=_trace, tmpdir=_tmpdir)
    out = np.stack([np.asarray(res.results[b]["out"], np.float32)
                    for b in range(B)])
    kernel.last_exec_time_ns = res.exec_time_ns
    return out


# revision 8
# speedup vs baseline: 1.1691x; 1.0096x over previous
"""Dual-RoPE attention block (B=8, S=1024, 16 heads x 64) on 8 NeuronCores.

v2: single fused pipeline so ScalarE (exp) starts ~130us earlier and all
engines stay busy end-to-end.  PSUM budget (8 banks): 2x scores tile
[128,1024]f32 (4 banks) + 2x PV tile [65,1024]f32 (4 banks); projection
chunks borrow the scores buffers in pass-tail windows.

Sharding: data-parallel over batch, one batch element per core.

Per-core dataflow (all matmuls bf16 inputs, fp32 PSUM accumulation):
  - qk-proj of head-pair 0 upfront; V-projection chunks interleaved with
    head-pair 0's first attention pass (chunk kc emitted just before the
    PV that consumes it); thereafter q-chunk of cc+1 projected in the tail
    of pass 0, k-chunk in the tail of pass 1.
  - scores row-tiled 2x (K=64; heads at partitions 0-63 / 64-127 run
    concurrently in the PE array).
  - exp on ScalarE; optionally a subset of key-chunks on VectorE via a
    bf16 Schraudolph bit-trick (tensor_scalar -> int16 -> bitcast bf16).
  - PV with the 65-row trick: vext column blocks are [2.0-const | v], so PV
    row 0 yields 2*sum_k(exp); rec = recip(2 sum) folds pass-averaging.
  - softmax without max-subtraction (scores O(10), exp safe in fp32).
  - rec broadcast via gpsimd.partition_broadcast; normalize on DVE in bf16.
"""

import numpy as np
import ml_dtypes

B, S, DM = 8, 1024, 1024
NH, HD = 16, 64
HD1 = HD + 1
NC = 8                # cores

# Schraudolph-on-DVE key-chunk assignment (kc values whose exp runs on
# VectorE instead of ScalarE). () disables.
DVE_KCS = ()
SCH_A = 184.6630
SCH_B = 16249.5

_CACHE = {}


def _build(dve_kcs=DVE_KCS):
    key = ("final", tuple(dve_kcs))
    if key in _CACHE:
        return _CACHE[key]
    from concourse import bacc, mybir
    import concourse.tile as tile

    f32 = mybir.dt.float32
    bf16 = mybir.dt.bfloat16
    i16 = mybir.dt.int16
    EXP = mybir.ActivationFunctionType.Exp
    MULT = mybir.AluOpType.mult
    ADD = mybir.AluOpType.add

    nc = bacc.Bacc("TRN2", target_bir_lowering=False, debug=False,
                   enable_asserts=False, num_devices=NC)

    xT_d = nc.dram_tensor("xT", [DM, S], bf16, kind="ExternalInput").ap()
    wqkT_d = nc.dram_tensor("wqkT", [DM, 2 * DM], bf16, kind="ExternalInput").ap()
    wqk0_d = nc.dram_tensor("wqk0", [128, 2048], bf16, kind="ExternalInput").ap()
    wvT_d = nc.dram_tensor("wvT", [DM, DM], bf16, kind="ExternalInput").ap()
    woT_d = nc.dram_tensor("woT", [DM, DM], bf16, kind="ExternalInput").ap()
    trig_d = nc.dram_tensor("trig", [128, 4 * S], bf16, kind="ExternalInput").ap()
    out_d = nc.dram_tensor("out", [S, DM], f32, kind="ExternalOutput").ap()

    with tile.TileContext(nc) as tc:
        with (
            tc.tile_pool(name="persist", bufs=1) as pp,
            tc.tile_pool(name="qkt", bufs=3) as qp,
            tc.tile_pool(name="expp", bufs=6) as ep,
            tc.tile_pool(name="smal", bufs=3) as sp,
            tc.tile_pool(name="bigps", bufs=2, space="PSUM") as bps,
            tc.tile_pool(name="pvps", bufs=1, space="PSUM") as pvp_pool,
        ):
            # ---------- persistent tiles + input DMA (ordered by need) -----
            xT_sb = [pp.tile([128, S], bf16, name=f"xT{i}") for i in range(8)]
            wqkT_sb = [pp.tile([128, 2 * DM], bf16, name=f"wqk{i}")
                       for i in range(8)]
            wvT_sb = [pp.tile([128, DM], bf16, name=f"wv{i}") for i in range(8)]
            woT_sb = [pp.tile([128, DM], bf16, name=f"woT{i}") for i in range(8)]
            trig_sb = pp.tile([128, 4 * S], bf16, name="trig")
            trigC_t = [trig_sb[:, p * S:(p + 1) * S] for p in range(2)]
            trigS_t = [trig_sb[:, (2 + p) * S:(3 + p) * S] for p in range(2)]
            vext = [pp.tile([128, NH * HD1], bf16, name=f"vext{i}")
                    for i in range(8)]
            attn_b = [pp.tile([128, S], bf16, name=f"attnb{i}") for i in range(8)]

            wqk0_sb = pp.tile([128, 2048], bf16, name="wqk0")
            # xT gates the first projection: split it across both HWDGE
            # queues ahead of everything except the tiny wqk0 slice.
            nc.sync.dma_start(wqk0_sb[:], wqk0_d[:])
            for i in range(8):
                eng = nc.sync if i % 2 == 0 else nc.scalar
                eng.dma_start(xT_sb[i][:], xT_d[i * 128:(i + 1) * 128, :])
            nc.scalar.dma_start(trig_sb[:], trig_d[:])
            # bulk weights stay off the SP queue: the rotate-half swap strips
            # (critical path of every rope) are issued there on demand
            for i in range(8):
                nc.gpsimd.dma_start(wvT_sb[i][:], wvT_d[i * 128:(i + 1) * 128, :])
            for i in range(8):
                nc.scalar.dma_start(wqkT_sb[i][:],
                                    wqkT_d[i * 128:(i + 1) * 128, :])
            for i in range(8):
                nc.gpsimd.dma_start(woT_sb[i][:], woT_d[i * 128:(i + 1) * 128, :])

            # roped q/k for both passes, double-buffered across cc:
            # roped[cc%2][pss][0]=q chunk, [1]=k chunk
            roped = [[[pp.tile([128, S], bf16, name=f"rope{par}_{p}_{t}")
                       for t in range(2)] for p in range(2)] for par in range(2)]

            def qk_proj_chunk(cc, which):
                """Project chunk `which` (0=q, 1=k) of head-pair cc into
                [c, s] layout and RoPE it for both passes."""
                wcol = cc + 8 * which
                ps = bps.tile([128, S], f32, tag="big", bufs=3,
                              name=f"qkps{cc}_{which}")
                for dc in range(8):
                    if cc == 0:
                        wsl = wqk0_sb[:, dc * 256 + which * 128:
                                      dc * 256 + (which + 1) * 128]
                    else:
                        wsl = wqkT_sb[dc][:, wcol * 128:(wcol + 1) * 128]
                    for n in range(2):
                        nc.tensor.matmul(
                            ps[:, n * 512:(n + 1) * 512],
                            wsl,
                            xT_sb[dc][:, n * 512:(n + 1) * 512],
                            start=(dc == 0), stop=(dc == 7))
                qk = qp.tile([128, S], bf16, tag="qk", bufs=2,
                             name=f"qk{cc}_{which}")
                nc.vector.tensor_copy(qk[:], ps[:])
                # rotate_half partner copy.  The head-dim partition
                # order is [d0-15, d32-47, d16-31, d48-63] (host-side
                # permutation; scores are invariant to it), so partners
                # sit in 16-row halves of each 32-partition quadrant:
                # expressible as a DVE stream_shuffle (used for pair 0,
                # where the DMA queues are still draining weights) or as
                # 16-row SBUF strips.
                sw = qp.tile([128, S], bf16, tag="sw", bufs=2,
                             name=f"sw{cc}_{which}")
                if cc == 0:
                    nc.vector.stream_shuffle(
                        sw[:], qk[:],
                        list(range(16, 32)) + list(range(16)))
                else:
                    for quad in range(4):
                        for f in range(2):
                            o0 = quad * 32 + f * 16
                            i0 = quad * 32 + (1 - f) * 16
                            nc.sync.dma_start(sw[o0:o0 + 16, :],
                                              qk[i0:i0 + 16, :])
                for pss in range(2):
                    a = qp.tile([128, S], bf16, tag="ropeA", bufs=1,
                                name=f"ropeA{cc}_{which}_{pss}")
                    bb = qp.tile([128, S], bf16, tag="ropeB", bufs=1,
                                 name=f"ropeB{cc}_{which}_{pss}")
                    nc.vector.tensor_mul(a[:], qk[:], trigC_t[pss][:])
                    nc.vector.tensor_mul(bb[:], sw[:], trigS_t[pss][:])
                    nc.vector.tensor_add(roped[cc % 2][pss][which][:],
                                         a[:], bb[:])

            def v_proj(sc):
                """Project V chunk sc (128 seq rows) into vext[sc] with the
                2.0-const column 0 per head."""
                ps = bps.tile([128, S], f32, tag="big", bufs=3,
                              name=f"vps{sc}")
                for dc in range(8):
                    for n in range(2):
                        nc.tensor.matmul(
                            ps[:, n * 512:(n + 1) * 512],
                            xT_sb[dc][:, sc * 128:(sc + 1) * 128],
                            wvT_sb[dc][:, n * 512:(n + 1) * 512],
                            start=(dc == 0), stop=(dc == 7))
                vv = vext[sc][:].rearrange("p (h e) -> p h e", e=HD1)
                nc.vector.tensor_copy(
                    vv[:, :, 1:HD1],
                    ps[:].rearrange("p (h e) -> p h e", e=HD))
                nc.vector.memset(vv[:, :, 0:1], 2.0)

            # Phase = (cc, pss, n-half): scores+exp of a phase run while
            # the PREVIOUS phase's PV accumulates underneath (per-kc
            # interleave).  PV psum is then 2x [65,512] (2 banks) instead
            # of 4, buying a THIRD scores buffer: projection insertions
            # hold one buf while two still rotate, so the exp stream on
            # ScalarE never starves.
            quarters = {}
            nquart = {}

            def finish_phase(pd, pvt):
                cc, pss, n, _ = pd
                hE, hO = 2 * cc, 2 * cc + 1
                for g, h in ((0, hE), (1, hO)):
                    pvp = pvt[g]
                    pv_sb = sp.tile([HD1, 512], bf16, tag="pvsb", bufs=4,
                                    name=f"pvsb{pss}_{h}_{n}")
                    nc.vector.tensor_copy(pv_sb[:], pvp[:])
                    recf = sp.tile([1, 512], f32, tag="recf", bufs=2,
                                   name=f"recf{pss}_{h}_{n}")
                    nc.vector.reciprocal_approx_fast(recf[0:1, :],
                                                     pvp[0:1, :])
                    rec = sp.tile([1, 512], bf16, tag="rec", bufs=2,
                                  name=f"rec{pss}_{h}_{n}")
                    with nc.allow_low_precision(
                            reason="bf16 recip of softmax sums"):
                        nc.vector.tensor_copy(rec[0:1, :], recf[0:1, :])
                    bc_sb = sp.tile([HD1, 512], bf16, tag="bcsb", bufs=2,
                                    name=f"bcsb{pss}_{h}_{n}")
                    nc.gpsimd.partition_broadcast(bc_sb[:, :], rec[0:1, :],
                                                  channels=HD1)
                    ct = sp.tile([HD1, 512], bf16, tag=f"ct{pss}{g}{n}",
                                 bufs=1, name=f"ct{pss}_{h}_{n}")
                    nc.vector.tensor_mul(ct[:], pv_sb[:], bc_sb[:])
                    quarters[(cc, pss, g, n)] = ct
                nquart[cc] = nquart.get(cc, 0) + 2
                if nquart[cc] == 8:
                    for g, h in ((0, hE), (1, hO)):
                        hh = (h % 2) * 64
                        ah = sp.tile([HD1, S], bf16, tag="ah", bufs=2,
                                     name=f"ah{h}")
                        for nn in range(2):
                            nc.vector.tensor_add(
                                ah[:, nn * 512:(nn + 1) * 512],
                                quarters.pop((cc, 0, g, nn))[:],
                                quarters.pop((cc, 1, g, nn))[:])
                        nc.sync.dma_start(attn_b[cc][hh:hh + 64, :],
                                          ah[1:HD1, :])

            def run_phase(cc, pss, n, pending, extras=None):
                q1 = roped[cc % 2][pss][0]
                k1 = roped[cc % 2][pss][1]
                pvt = None
                if pending is not None:
                    pcc = pending[0]
                    pvt = [pvp_pool.tile([HD1, 512], f32, tag="pv", bufs=2,
                                         name=f"pv{pcc}_{g}")
                           for g in range(2)]
                es_list = []
                for kc in range(8):
                    scp = bps.tile([128, S], f32, tag="big", bufs=3,
                                   name=f"scp{pss}_{cc}_{kc}_{n}")
                    for g, hh in ((0, 0), (1, 64)):
                        nc.tensor.matmul(
                            scp[:, g * 512:(g + 1) * 512],
                            k1[hh:hh + 64, kc * 128:(kc + 1) * 128],
                            q1[hh:hh + 64, n * 512:(n + 1) * 512],
                            start=True, stop=True)
                    es = ep.tile([128, S], bf16, tag="expS", bufs=12,
                                 name=f"es{pss}_{cc}_{kc}_{n}")
                    nc.scalar.activation(es[:], scp[:], EXP, scale=0.125)
                    if extras and kc in extras:
                        extras[kc]()
                    if pending is not None:
                        pcc, ppss, pn, pes = pending
                        for g, h in ((0, 2 * pcc), (1, 2 * pcc + 1)):
                            nc.tensor.matmul(
                                pvt[g][:, :],
                                vext[kc][:, h * HD1:(h + 1) * HD1],
                                pes[kc][:, g * 512:(g + 1) * 512],
                                start=(kc == 0), stop=(kc == 7))
                    es_list.append(es)
                if pending is not None:
                    finish_phase(pending, pvt)
                return (cc, pss, n, es_list)

            # ---------- fused pipeline ----------
            qk_proj_chunk(0, 0)
            qk_proj_chunk(0, 1)
            v_proj(0)
            v_proj(1)

            pending = None
            for cc in range(8):
                for pss in range(2):
                    for n in range(2):
                        ex = None
                        if cc == 0 and pss == 0:
                            ex = {kc: (lambda s=3 * n + kc: v_proj(s))
                                  for kc in range(3 if n == 0 else 3)}
                            ex = {kc: (lambda s=2 + 3 * n + kc: v_proj(s))
                                  for kc in range(3)}
                        elif cc < 7 and pss == 1 and n == 0:
                            ex = {0: (lambda c=cc: qk_proj_chunk(c + 1, 0)),
                                  4: (lambda c=cc: qk_proj_chunk(c + 1, 1))}
                        pending = run_phase(cc, pss, n, pending, extras=ex)
            # drain the last phase's PV
            pcc = pending[0]
            pvt = [pvp_pool.tile([HD1, 512], f32, tag="pv", bufs=2,
                                 name=f"pvD_{g}") for g in range(2)]
            for kc in range(8):
                pes = pending[3]
                for g, h in ((0, 2 * pcc), (1, 2 * pcc + 1)):
                    nc.tensor.matmul(
                        pvt[g][:, :], vext[kc][:, h * HD1:(h + 1) * HD1],
                        pes[kc][:, g * 512:(g + 1) * 512],
                        start=(kc == 0), stop=(kc == 7))
            finish_phase(pending, pvt)

            # ---------- output projection ----------
            # partial accumulation over heads 0-6 of the next sc chunk is
            # emitted before the cc=7 finisher of the current one, so the PE
            # works while the last head-pair's normalize chain completes.
            def oproj_partial(sc, op):
                for cc in range(7):
                    for n in range(2):
                        nc.tensor.matmul(
                            op[:, n * 512:(n + 1) * 512],
                            attn_b[cc][:, sc * 128:(sc + 1) * 128],
                            woT_sb[cc][:, n * 512:(n + 1) * 512],
                            start=(cc == 0), stop=False)

            def oproj_finish(sc, op):
                for n in range(2):
                    nc.tensor.matmul(
                        op[:, n * 512:(n + 1) * 512],
                        attn_b[7][:, sc * 128:(sc + 1) * 128],
                        woT_sb[7][:, n * 512:(n + 1) * 512],
                        start=False, stop=True)
                ob = sp.tile([128, DM], f32, tag="ob", bufs=2,
                             name=f"ob{sc}")
                nc.vector.tensor_copy(ob[:], op[:])
                eng = nc.sync if sc % 2 == 0 else nc.scalar
                eng.dma_start(out_d[sc * 128:(sc + 1) * 128, :], ob[:])

            ops = {}
            ops[0] = bps.tile([128, DM], f32, tag="big", bufs=3, name="op0")
            oproj_partial(0, ops[0])
            for sc in range(8):
                if sc + 1 < 8:
                    ops[sc + 1] = bps.tile([128, DM], f32, tag="big",
                                           bufs=3, name=f"op{sc + 1}")
                    oproj_partial(sc + 1, ops[sc + 1])
                oproj_finish(sc, ops.pop(sc))

    nc.compile()
    _CACHE[key] = nc
    return nc


def _prep_inputs(hidden_states, cos, sin, w_qkv, w_o):
    bf = ml_dtypes.bfloat16
    xT = np.ascontiguousarray(
        hidden_states.transpose(0, 2, 1)).astype(bf)          # [B, DM, S]
    wqkT = np.ascontiguousarray(w_qkv[:2 * DM].T).astype(bf)  # [DM, 2DM]
    wvT = np.ascontiguousarray(w_qkv[2 * DM:].T).astype(bf)   # [DM, DM]
    woT = np.ascontiguousarray(w_o.T).astype(bf)              # [DM, DM]

    idx = np.arange(S).reshape(32, 32).T.reshape(-1)
    perm64 = np.concatenate([np.arange(0, 16), np.arange(32, 48),
                             np.arange(16, 32), np.arange(48, 64)])
    # permute every 64-wide head block of the q/k weight columns
    wqk_pc = wqkT.reshape(DM, 32, 64)[:, :, perm64].reshape(DM, 2 * DM)
    wqkT = np.ascontiguousarray(wqk_pc)
    d = perm64[np.arange(128) % HD]
    sign = np.where(d < 32, -1.0, 1.0).astype(np.float32)
    trig = np.concatenate([
        cos[:, d].T, cos[idx][:, d].T,
        sin[:, d].T * sign[:, None], sin[idx][:, d].T * sign[:, None],
    ], axis=1).astype(bf)                                     # [128, 4S]
    wqk0 = np.ascontiguousarray(
        np.concatenate([wqkT[:, 0:128], wqkT[:, 1024:1152]], axis=1)
        .reshape(8, 128, 256).transpose(1, 0, 2).reshape(128, 2048))
    shared = {"wqkT": wqkT, "wqk0": wqk0, "wvT": wvT, "woT": woT,
              "trig": np.ascontiguousarray(trig)}
    return [{"xT": np.ascontiguousarray(xT[b]), **shared} for b in range(B)]


def _install_ntff_hook():
    import sys, types
    if "antenv.axon_hooks" in sys.modules:
        return
    try:
        from trn_agent_boot.trn_boot import _ntff_profile_via_ctypes
        hook = _ntff_profile_via_ctypes('/opt/axon/libaxon_pjrt.so')
    except Exception:
        hook = None
    mod = types.ModuleType("antenv.axon_hooks")
    mod.get_axon_ntff_profile_hook = lambda: hook
    mod.set_axon_ntff_profile_hook = lambda h: None
    sys.modules["antenv.axon_hooks"] = mod


def kernel(hidden_states, cos, sin, w_qkv, w_o, _trace=False, _tmpdir=None):
    from concourse import bass_utils
    if _trace:
        _install_ntff_hook()
    nc = _build()
    in_maps = _prep_inputs(np.asarray(hidden_states, np.float32),
                           np.asarray(cos, np.float32),
                           np.asarray(sin, np.float32),
                           np.asarray(w_qkv, np.float32),
                           np.asarray(w_o, np.float32))
    res = bass_utils.run_bass_kernel_spmd(
        nc, in_maps, core_ids=list(range(NC)),
        trace=_trace, tmpdir=_tmpdir)
    out = np.stack([np.asarray(res.results[b]["out"], np.float32)
                    for b in range(B)])
    kernel.last_exec_time_ns = res.exec_time_ns
    return out


# revision 9
# speedup vs baseline: 1.1735x; 1.0038x over previous
"""Dual-RoPE attention block (B=8, S=1024, 16 heads x 64) on 8 NeuronCores.

v2: single fused pipeline so ScalarE (exp) starts ~130us earlier and all
engines stay busy end-to-end.  PSUM budget (8 banks): 2x scores tile
[128,1024]f32 (4 banks) + 2x PV tile [65,1024]f32 (4 banks); projection
chunks borrow the scores buffers in pass-tail windows.

Sharding: data-parallel over batch, one batch element per core.

Per-core dataflow (all matmuls bf16 inputs, fp32 PSUM accumulation):
  - qk-proj of head-pair 0 upfront; V-projection chunks interleaved with
    head-pair 0's first attention pass (chunk kc emitted just before the
    PV that consumes it); thereafter q-chunk of cc+1 projected in the tail
    of pass 0, k-chunk in the tail of pass 1.
  - scores row-tiled 2x (K=64; heads at partitions 0-63 / 64-127 run
    concurrently in the PE array).
  - exp on ScalarE; optionally a subset of key-chunks on VectorE via a
    bf16 Schraudolph bit-trick (tensor_scalar -> int16 -> bitcast bf16).
  - PV with the 65-row trick: vext column blocks are [2.0-const | v], so PV
    row 0 yields 2*sum_k(exp); rec = recip(2 sum) folds pass-averaging.
  - softmax without max-subtraction (scores O(10), exp safe in fp32).
  - rec broadcast via gpsimd.partition_broadcast; normalize on DVE in bf16.
"""

import numpy as np
import ml_dtypes

B, S, DM = 8, 1024, 1024
NH, HD = 16, 64
HD1 = HD + 1
NC = 8                # cores

# Schraudolph-on-DVE key-chunk assignment (kc values whose exp runs on
# VectorE instead of ScalarE). () disables.
DVE_KCS = ()
SCH_A = 184.6630
SCH_B = 16249.5

_CACHE = {}


def _build(dve_kcs=DVE_KCS):
    key = ("final", tuple(dve_kcs))
    if key in _CACHE:
        return _CACHE[key]
    from concourse import bacc, mybir
    import concourse.tile as tile

    f32 = mybir.dt.float32
    bf16 = mybir.dt.bfloat16
    i16 = mybir.dt.int16
    EXP = mybir.ActivationFunctionType.Exp
    MULT = mybir.AluOpType.mult
    ADD = mybir.AluOpType.add

    nc = bacc.Bacc("TRN2", target_bir_lowering=False, debug=False,
                   enable_asserts=False, num_devices=NC)

    xT_d = nc.dram_tensor("xT", [DM, S], bf16, kind="ExternalInput").ap()
    wqkT_d = nc.dram_tensor("wqkT", [DM, 2 * DM], bf16, kind="ExternalInput").ap()
    wqk0_d = nc.dram_tensor("wqk0", [128, 2048], bf16, kind="ExternalInput").ap()
    wvT_d = nc.dram_tensor("wvT", [DM, DM], bf16, kind="ExternalInput").ap()
    woT_d = nc.dram_tensor("woT", [DM, DM], bf16, kind="ExternalInput").ap()
    trig_d = nc.dram_tensor("trig", [128, 4 * S], bf16, kind="ExternalInput").ap()
    out_d = nc.dram_tensor("out", [S, DM], f32, kind="ExternalOutput").ap()

    with tile.TileContext(nc) as tc:
        with (
            tc.tile_pool(name="persist", bufs=1) as pp,
            tc.tile_pool(name="qkt", bufs=3) as qp,
            tc.tile_pool(name="expp", bufs=6) as ep,
            tc.tile_pool(name="smal", bufs=3) as sp,
            tc.tile_pool(name="bigps", bufs=2, space="PSUM") as bps,
            tc.tile_pool(name="pvps", bufs=1, space="PSUM") as pvp_pool,
        ):
            # ---------- persistent tiles + input DMA (ordered by need) -----
            xT_sb = [pp.tile([128, S], bf16, name=f"xT{i}") for i in range(8)]
            wqkT_sb = [pp.tile([128, 2 * DM], bf16, name=f"wqk{i}")
                       for i in range(8)]
            wvT_sb = [pp.tile([128, DM], bf16, name=f"wv{i}") for i in range(8)]
            woT_sb = [pp.tile([128, DM], bf16, name=f"woT{i}") for i in range(8)]
            trig_sb = pp.tile([128, 4 * S], bf16, name="trig")
            trigC_t = [trig_sb[:, p * S:(p + 1) * S] for p in range(2)]
            trigS_t = [trig_sb[:, (2 + p) * S:(3 + p) * S] for p in range(2)]
            vext = [pp.tile([128, NH * HD1], bf16, name=f"vext{i}")
                    for i in range(8)]
            attn_b = [pp.tile([128, S], bf16, name=f"attnb{i}") for i in range(8)]

            wqk0_sb = pp.tile([128, 2048], bf16, name="wqk0")
            # xT gates the first projection: split it across both HWDGE
            # queues ahead of everything except the tiny wqk0 slice.
            nc.sync.dma_start(wqk0_sb[:], wqk0_d[:])
            for i in range(8):
                eng = nc.sync if i % 2 == 0 else nc.scalar
                eng.dma_start(xT_sb[i][:], xT_d[i * 128:(i + 1) * 128, :])
            nc.scalar.dma_start(trig_sb[:], trig_d[:])
            # bulk weights stay off the SP queue: the rotate-half swap strips
            # (critical path of every rope) are issued there on demand
            for i in range(8):
                nc.gpsimd.dma_start(wvT_sb[i][:], wvT_d[i * 128:(i + 1) * 128, :])
            for i in range(8):
                nc.scalar.dma_start(wqkT_sb[i][:],
                                    wqkT_d[i * 128:(i + 1) * 128, :])
            for i in range(8):
                nc.gpsimd.dma_start(woT_sb[i][:], woT_d[i * 128:(i + 1) * 128, :])

            # roped q/k for both passes, double-buffered across cc:
            # roped[cc%2][pss][0]=q chunk, [1]=k chunk
            roped = [[[pp.tile([128, S], bf16, name=f"rope{par}_{p}_{t}")
                       for t in range(2)] for p in range(2)] for par in range(2)]

            def qk_proj_chunk(cc, which):
                """Project chunk `which` (0=q, 1=k) of head-pair cc into
                [c, s] layout and RoPE it for both passes."""
                wcol = cc + 8 * which
                ps = bps.tile([128, S], f32, tag="big", bufs=3,
                              name=f"qkps{cc}_{which}")
                for dc in range(8):
                    if cc == 0:
                        wsl = wqk0_sb[:, dc * 256 + which * 128:
                                      dc * 256 + (which + 1) * 128]
                    else:
                        wsl = wqkT_sb[dc][:, wcol * 128:(wcol + 1) * 128]
                    for n in range(2):
                        nc.tensor.matmul(
                            ps[:, n * 512:(n + 1) * 512],
                            wsl,
                            xT_sb[dc][:, n * 512:(n + 1) * 512],
                            start=(dc == 0), stop=(dc == 7))
                qk = qp.tile([128, S], bf16, tag="qk", bufs=2,
                             name=f"qk{cc}_{which}")
                nc.vector.tensor_copy(qk[:], ps[:])
                # rotate_half partner copy.  The head-dim partition
                # order is [d0-15, d32-47, d16-31, d48-63] (host-side
                # permutation; scores are invariant to it), so partners
                # sit in 16-row halves of each 32-partition quadrant:
                # expressible as a DVE stream_shuffle (used for pair 0,
                # where the DMA queues are still draining weights) or as
                # 16-row SBUF strips.
                sw = qp.tile([128, S], bf16, tag="sw", bufs=2,
                             name=f"sw{cc}_{which}")
                if cc == 0:
                    nc.vector.stream_shuffle(
                        sw[:], qk[:],
                        list(range(16, 32)) + list(range(16)))
                else:
                    for quad in range(4):
                        for f in range(2):
                            o0 = quad * 32 + f * 16
                            i0 = quad * 32 + (1 - f) * 16
                            nc.sync.dma_start(sw[o0:o0 + 16, :],
                                              qk[i0:i0 + 16, :])
                for pss in range(2):
                    a = qp.tile([128, S], bf16, tag="ropeA", bufs=1,
                                name=f"ropeA{cc}_{which}_{pss}")
                    bb = qp.tile([128, S], bf16, tag="ropeB", bufs=1,
                                 name=f"ropeB{cc}_{which}_{pss}")
                    nc.vector.tensor_mul(a[:], qk[:], trigC_t[pss][:])
                    nc.vector.tensor_mul(bb[:], sw[:], trigS_t[pss][:])
                    nc.vector.tensor_add(roped[cc % 2][pss][which][:],
                                         a[:], bb[:])

            def v_proj(sc):
                """Project V chunk sc (128 seq rows) into vext[sc] with the
                2.0-const column 0 per head."""
                ps = bps.tile([128, S], f32, tag="big", bufs=3,
                              name=f"vps{sc}")
                for dc in range(8):
                    for n in range(2):
                        nc.tensor.matmul(
                            ps[:, n * 512:(n + 1) * 512],
                            xT_sb[dc][:, sc * 128:(sc + 1) * 128],
                            wvT_sb[dc][:, n * 512:(n + 1) * 512],
                            start=(dc == 0), stop=(dc == 7))
                vv = vext[sc][:].rearrange("p (h e) -> p h e", e=HD1)
                nc.vector.tensor_copy(
                    vv[:, :, 1:HD1],
                    ps[:].rearrange("p (h e) -> p h e", e=HD))
                nc.vector.memset(vv[:, :, 0:1], 2.0)

            # Phase = (cc, pss, n-half): scores+exp of a phase run while
            # the PREVIOUS phase's PV accumulates underneath (per-kc
            # interleave).  PV psum is then 2x [65,512] (2 banks) instead
            # of 4, buying a THIRD scores buffer: projection insertions
            # hold one buf while two still rotate, so the exp stream on
            # ScalarE never starves.
            quarters = {}
            nquart = {}

            def finish_phase(pd, pvt):
                cc, pss, n, _ = pd
                hE, hO = 2 * cc, 2 * cc + 1
                for g, h in ((0, hE), (1, hO)):
                    pvp = pvt[g]
                    pv_sb = sp.tile([HD1, 512], bf16, tag="pvsb", bufs=4,
                                    name=f"pvsb{pss}_{h}_{n}")
                    nc.vector.tensor_copy(pv_sb[:], pvp[:])
                    recf = sp.tile([1, 512], f32, tag="recf", bufs=2,
                                   name=f"recf{pss}_{h}_{n}")
                    nc.vector.reciprocal_approx_fast(recf[0:1, :],
                                                     pvp[0:1, :])
                    rec = sp.tile([1, 512], bf16, tag="rec", bufs=2,
                                  name=f"rec{pss}_{h}_{n}")
                    with nc.allow_low_precision(
                            reason="bf16 recip of softmax sums"):
                        nc.vector.tensor_copy(rec[0:1, :], recf[0:1, :])
                    bc_sb = sp.tile([HD1, 512], bf16, tag="bcsb", bufs=2,
                                    name=f"bcsb{pss}_{h}_{n}")
                    nc.gpsimd.partition_broadcast(bc_sb[:, :], rec[0:1, :],
                                                  channels=HD1)
                    ct = sp.tile([HD1, 512], bf16, tag=f"ct{pss}{g}{n}",
                                 bufs=1, name=f"ct{pss}_{h}_{n}")
                    nc.vector.tensor_mul(ct[:], pv_sb[:], bc_sb[:])
                    quarters[(cc, pss, g, n)] = ct
                nquart[cc] = nquart.get(cc, 0) + 2
                if nquart[cc] == 8:
                    for g, h in ((0, hE), (1, hO)):
                        hh = (h % 2) * 64
                        ah = sp.tile([HD1, S], bf16, tag="ah", bufs=2,
                                     name=f"ah{h}")
                        for nn in range(2):
                            nc.vector.tensor_add(
                                ah[:, nn * 512:(nn + 1) * 512],
                                quarters.pop((cc, 0, g, nn))[:],
                                quarters.pop((cc, 1, g, nn))[:])
                        nc.sync.dma_start(attn_b[cc][hh:hh + 64, :],
                                          ah[1:HD1, :])

            def run_phase(cc, pss, n, pending, extras=None):
                q1 = roped[cc % 2][pss][0]
                k1 = roped[cc % 2][pss][1]
                pvt = None
                if pending is not None:
                    pcc = pending[0]
                    pvt = [pvp_pool.tile([HD1, 512], f32, tag="pv", bufs=2,
                                         name=f"pv{pcc}_{g}")
                           for g in range(2)]
                es_list = []
                for kc in range(8):
                    scp = bps.tile([128, S], f32, tag="big", bufs=3,
                                   name=f"scp{pss}_{cc}_{kc}_{n}")
                    for g, hh in ((0, 0), (1, 64)):
                        nc.tensor.matmul(
                            scp[:, g * 512:(g + 1) * 512],
                            k1[hh:hh + 64, kc * 128:(kc + 1) * 128],
                            q1[hh:hh + 64, n * 512:(n + 1) * 512],
                            start=True, stop=True)
                    es = ep.tile([128, S], bf16, tag="expS", bufs=12,
                                 name=f"es{pss}_{cc}_{kc}_{n}")
                    nc.scalar.activation(es[:], scp[:], EXP, scale=0.125)
                    if extras and kc in extras:
                        extras[kc]()
                    if pending is not None:
                        pcc, ppss, pn, pes = pending
                        for g, h in ((0, 2 * pcc), (1, 2 * pcc + 1)):
                            nc.tensor.matmul(
                                pvt[g][:, :],
                                vext[kc][:, h * HD1:(h + 1) * HD1],
                                pes[kc][:, g * 512:(g + 1) * 512],
                                start=(kc == 0), stop=(kc == 7))
                    es_list.append(es)
                if pending is not None:
                    finish_phase(pending, pvt)
                return (cc, pss, n, es_list)

            # ---------- fused pipeline ----------
            # head-pair 0 prologue: emit both chunks' pass-0 rope before
            # any pass-1 rope so the first scores aren't FIFO-delayed
            cc0_parts = []
            for which in range(2):
                ps = bps.tile([128, S], f32, tag="big", bufs=3,
                              name=f"qkps0_{which}")
                qk_w = None
                for dc in range(8):
                    wsl = wqk0_sb[:, dc * 256 + which * 128:
                                  dc * 256 + (which + 1) * 128]
                    for n in range(2):
                        nc.tensor.matmul(
                            ps[:, n * 512:(n + 1) * 512], wsl,
                            xT_sb[dc][:, n * 512:(n + 1) * 512],
                            start=(dc == 0), stop=(dc == 7))
                qk = qp.tile([128, S], bf16, tag="qk", bufs=2,
                             name=f"qk0_{which}")
                nc.vector.tensor_copy(qk[:], ps[:])
                sw = qp.tile([128, S], bf16, tag="sw", bufs=2,
                             name=f"sw0_{which}")
                nc.vector.stream_shuffle(
                    sw[:], qk[:], list(range(16, 32)) + list(range(16)))
                cc0_parts.append((qk, sw))
            for pss in range(2):
                for which in range(2):
                    qk, sw = cc0_parts[which]
                    a = qp.tile([128, S], bf16, tag="ropeA", bufs=1,
                                name=f"ropeA0_{which}_{pss}")
                    bb = qp.tile([128, S], bf16, tag="ropeB", bufs=1,
                                 name=f"ropeB0_{which}_{pss}")
                    nc.vector.tensor_mul(a[:], qk[:], trigC_t[pss][:])
                    nc.vector.tensor_mul(bb[:], sw[:], trigS_t[pss][:])
                    nc.vector.tensor_add(roped[0][pss][which][:],
                                         a[:], bb[:])
            v_proj(0)
            v_proj(1)

            pending = None
            for cc in range(8):
                for pss in range(2):
                    for n in range(2):
                        ex = None
                        if cc == 0 and pss == 0:
                            ex = {kc: (lambda s=3 * n + kc: v_proj(s))
                                  for kc in range(3 if n == 0 else 3)}
                            ex = {kc: (lambda s=2 + 3 * n + kc: v_proj(s))
                                  for kc in range(3)}
                        elif cc < 7 and pss == 1 and n == 0:
                            ex = {0: (lambda c=cc: qk_proj_chunk(c + 1, 0)),
                                  4: (lambda c=cc: qk_proj_chunk(c + 1, 1))}
                        pending = run_phase(cc, pss, n, pending, extras=ex)
            # drain the last phase's PV
            pcc = pending[0]
            pvt = [pvp_pool.tile([HD1, 512], f32, tag="pv", bufs=2,
                                 name=f"pvD_{g}") for g in range(2)]
            for kc in range(8):
                pes = pending[3]
                for g, h in ((0, 2 * pcc), (1, 2 * pcc + 1)):
                    nc.tensor.matmul(
                        pvt[g][:, :], vext[kc][:, h * HD1:(h + 1) * HD1],
                        pes[kc][:, g * 512:(g + 1) * 512],
                        start=(kc == 0), stop=(kc == 7))
            finish_phase(pending, pvt)

            # ---------- output projection ----------
            # partial accumulation over heads 0-6 of the next sc chunk is
            # emitted before the cc=7 finisher of the current one, so the PE
            # works while the last head-pair's normalize chain completes.
            def oproj_partial(sc, op):
                for cc in range(7):
                    for n in range(2):
                        nc.tensor.matmul(
                            op[:, n * 512:(n + 1) * 512],
                            attn_b[cc][:, sc * 128:(sc + 1) * 128],
                            woT_sb[cc][:, n * 512:(n + 1) * 512],
                            start=(cc == 0), stop=False)

            def oproj_finish(sc, op):
                for n in range(2):
                    nc.tensor.matmul(
                        op[:, n * 512:(n + 1) * 512],
                        attn_b[7][:, sc * 128:(sc + 1) * 128],
                        woT_sb[7][:, n * 512:(n + 1) * 512],
                        start=False, stop=True)
                ob = sp.tile([128, DM], f32, tag="ob", bufs=2,
                             name=f"ob{sc}")
                nc.vector.tensor_copy(ob[:], op[:])
                eng = nc.sync if sc % 2 == 0 else nc.scalar
                eng.dma_start(out_d[sc * 128:(sc + 1) * 128, :], ob[:])

            ops = {}
            ops[0] = bps.tile([128, DM], f32, tag="big", bufs=3, name="op0")
            oproj_partial(0, ops[0])
            for sc in range(8):
                if sc + 1 < 8:
                    ops[sc + 1] = bps.tile([128, DM], f32, tag="big",
                                           bufs=3, name=f"op{sc + 1}")
                    oproj_partial(sc + 1, ops[sc + 1])
                oproj_finish(sc, ops.pop(sc))

    nc.compile()
    _CACHE[key] = nc
    return nc


def _prep_inputs(hidden_states, cos, sin, w_qkv, w_o):
    bf = ml_dtypes.bfloat16
    xT = np.ascontiguousarray(
        hidden_states.transpose(0, 2, 1)).astype(bf)          # [B, DM, S]
    wqkT = np.ascontiguousarray(w_qkv[:2 * DM].T).astype(bf)  # [DM, 2DM]
    wvT = np.ascontiguousarray(w_qkv[2 * DM:].T).astype(bf)   # [DM, DM]
    woT = np.ascontiguousarray(w_o.T).astype(bf)              # [DM, DM]

    idx = np.arange(S).reshape(32, 32).T.reshape(-1)
    perm64 = np.concatenate([np.arange(0, 16), np.arange(32, 48),
                             np.arange(16, 32), np.arange(48, 64)])
    # permute every 64-wide head block of the q/k weight columns
    wqk_pc = wqkT.reshape(DM, 32, 64)[:, :, perm64].reshape(DM, 2 * DM)
    wqkT = np.ascontiguousarray(wqk_pc)
    d = perm64[np.arange(128) % HD]
    sign = np.where(d < 32, -1.0, 1.0).astype(np.float32)
    trig = np.concatenate([
        cos[:, d].T, cos[idx][:, d].T,
        sin[:, d].T * sign[:, None], sin[idx][:, d].T * sign[:, None],
    ], axis=1).astype(bf)                                     # [128, 4S]
    wqk0 = np.ascontiguousarray(
        np.concatenate([wqkT[:, 0:128], wqkT[:, 1024:1152]], axis=1)
        .reshape(8, 128, 256).transpose(1, 0, 2).reshape(128, 2048))
    shared = {"wqkT": wqkT, "wqk0": wqk0, "wvT": wvT, "woT": woT,
              "trig": np.ascontiguousarray(trig)}
    return [{"xT": np.ascontiguousarray(xT[b]), **shared} for b in range(B)]


def _install_ntff_hook():
    import sys, types
    if "antenv.axon_hooks" in sys.modules:
        return
    try:
        from trn_agent_boot.trn_boot import _ntff_profile_via_ctypes
        hook = _ntff_profile_via_ctypes('/opt/axon/libaxon_pjrt.so')
    except Exception:
        hook = None
    mod = types.ModuleType("antenv.axon_hooks")
    mod.get_axon_ntff_profile_hook = lambda: hook
    mod.set_axon_ntff_profile_hook = lambda h: None
    sys.modules["antenv.axon_hooks"] = mod


def kernel(hidden_states, cos, sin, w_qkv, w_o, _trace=False, _tmpdir=None):
    from concourse import bass_utils
    if _trace:
        _install_ntff_hook()
    nc = _build()
    in_maps = _prep_inputs(np.asarray(hidden_states, np.float32),
                           np.asarray(cos, np.float32),
                           np.asarray(sin, np.float32),
                           np.asarray(w_qkv, np.float32),
                           np.asarray(w_o, np.float32))
    res = bass_utils.run_bass_kernel_spmd(
        nc, in_maps, core_ids=list(range(NC)),
        trace=_trace, tmpdir=_tmpdir)
    out = np.stack([np.asarray(res.results[b]["out"], np.float32)
                    for b in range(B)])
    kernel.last_exec_time_ns = res.exec_time_ns
    return out


# revision 10
# speedup vs baseline: 1.1740x; 1.0004x over previous
"""Dual-RoPE attention block (B=8, S=1024, 16 heads x 64) on 8 NeuronCores.

v2: single fused pipeline so ScalarE (exp) starts ~130us earlier and all
engines stay busy end-to-end.  PSUM budget (8 banks): 2x scores tile
[128,1024]f32 (4 banks) + 2x PV tile [65,1024]f32 (4 banks); projection
chunks borrow the scores buffers in pass-tail windows.

Sharding: data-parallel over batch, one batch element per core.

Per-core dataflow (all matmuls bf16 inputs, fp32 PSUM accumulation):
  - qk-proj of head-pair 0 upfront; V-projection chunks interleaved with
    head-pair 0's first attention pass (chunk kc emitted just before the
    PV that consumes it); thereafter q-chunk of cc+1 projected in the tail
    of pass 0, k-chunk in the tail of pass 1.
  - scores row-tiled 2x (K=64; heads at partitions 0-63 / 64-127 run
    concurrently in the PE array).
  - exp on ScalarE; optionally a subset of key-chunks on VectorE via a
    bf16 Schraudolph bit-trick (tensor_scalar -> int16 -> bitcast bf16).
  - PV with the 65-row trick: vext column blocks are [2.0-const | v], so PV
    row 0 yields 2*sum_k(exp); rec = recip(2 sum) folds pass-averaging.
  - softmax without max-subtraction (scores O(10), exp safe in fp32).
  - rec broadcast via gpsimd.partition_broadcast; normalize on DVE in bf16.
"""

import numpy as np
import ml_dtypes

B, S, DM = 8, 1024, 1024
NH, HD = 16, 64
HD1 = HD + 1
NC = 8                # cores

# Schraudolph-on-DVE key-chunk assignment (kc values whose exp runs on
# VectorE instead of ScalarE). () disables.
DVE_KCS = ()
SCH_A = 184.6630
SCH_B = 16249.5

_CACHE = {}


def _build(dve_kcs=DVE_KCS):
    key = ("final", tuple(dve_kcs))
    if key in _CACHE:
        return _CACHE[key]
    from concourse import bacc, mybir
    import concourse.tile as tile

    f32 = mybir.dt.float32
    bf16 = mybir.dt.bfloat16
    i16 = mybir.dt.int16
    EXP = mybir.ActivationFunctionType.Exp
    MULT = mybir.AluOpType.mult
    ADD = mybir.AluOpType.add

    nc = bacc.Bacc("TRN2", target_bir_lowering=False, debug=False,
                   enable_asserts=False, num_devices=NC)

    xT_d = nc.dram_tensor("xT", [DM, S], bf16, kind="ExternalInput").ap()
    wqkT_d = nc.dram_tensor("wqkT", [DM, 2 * DM], bf16, kind="ExternalInput").ap()
    wqk0_d = nc.dram_tensor("wqk0", [128, 2048], bf16, kind="ExternalInput").ap()
    wvT_d = nc.dram_tensor("wvT", [DM, DM], bf16, kind="ExternalInput").ap()
    woT_d = nc.dram_tensor("woT", [DM, DM], bf16, kind="ExternalInput").ap()
    trig_d = nc.dram_tensor("trig", [128, 4 * S], bf16, kind="ExternalInput").ap()
    out_d = nc.dram_tensor("out", [S, DM], f32, kind="ExternalOutput").ap()

    with tile.TileContext(nc) as tc:
        with (
            tc.tile_pool(name="persist", bufs=1) as pp,
            tc.tile_pool(name="qkt", bufs=3) as qp,
            tc.tile_pool(name="expp", bufs=6) as ep,
            tc.tile_pool(name="smal", bufs=3) as sp,
            tc.tile_pool(name="bigps", bufs=2, space="PSUM") as bps,
            tc.tile_pool(name="pvps", bufs=1, space="PSUM") as pvp_pool,
        ):
            # ---------- persistent tiles + input DMA (ordered by need) -----
            xT_sb = [pp.tile([128, S], bf16, name=f"xT{i}") for i in range(8)]
            wqkT_sb = [pp.tile([128, 2 * DM], bf16, name=f"wqk{i}")
                       for i in range(8)]
            wvT_sb = [pp.tile([128, DM], bf16, name=f"wv{i}") for i in range(8)]
            woT_sb = [pp.tile([128, DM], bf16, name=f"woT{i}") for i in range(8)]
            trig_sb = pp.tile([128, 4 * S], bf16, name="trig")
            trigC_t = [trig_sb[:, p * S:(p + 1) * S] for p in range(2)]
            trigS_t = [trig_sb[:, (2 + p) * S:(3 + p) * S] for p in range(2)]
            vext = [pp.tile([128, NH * HD1], bf16, name=f"vext{i}")
                    for i in range(8)]
            attn_b = [pp.tile([128, S], bf16, name=f"attnb{i}") for i in range(8)]

            wqk0_sb = pp.tile([128, 2048], bf16, name="wqk0")
            # xT gates the first projection: split it across both HWDGE
            # queues ahead of everything except the tiny wqk0 slice.
            nc.sync.dma_start(wqk0_sb[:], wqk0_d[:])
            for i in range(8):
                eng = nc.sync if i % 2 == 0 else nc.scalar
                eng.dma_start(xT_sb[i][:], xT_d[i * 128:(i + 1) * 128, :])
            nc.scalar.dma_start(trig_sb[:], trig_d[:])
            # bulk weights stay off the SP queue: the rotate-half swap strips
            # (critical path of every rope) are issued there on demand
            for i in range(8):
                nc.gpsimd.dma_start(wvT_sb[i][:], wvT_d[i * 128:(i + 1) * 128, :])
            for i in range(8):
                nc.scalar.dma_start(wqkT_sb[i][:],
                                    wqkT_d[i * 128:(i + 1) * 128, :])
            for i in range(8):
                nc.gpsimd.dma_start(woT_sb[i][:], woT_d[i * 128:(i + 1) * 128, :])

            # roped q/k for both passes, double-buffered across cc:
            # roped[cc%2][pss][0]=q chunk, [1]=k chunk
            roped = [[[pp.tile([128, S], bf16, name=f"rope{par}_{p}_{t}")
                       for t in range(2)] for p in range(2)] for par in range(2)]

            def qk_proj_chunk(cc, which):
                """Project chunk `which` (0=q, 1=k) of head-pair cc into
                [c, s] layout and RoPE it for both passes."""
                wcol = cc + 8 * which
                ps = bps.tile([128, S], f32, tag="big", bufs=3,
                              name=f"qkps{cc}_{which}")
                for dc in range(8):
                    if cc == 0:
                        wsl = wqk0_sb[:, dc * 256 + which * 128:
                                      dc * 256 + (which + 1) * 128]
                    else:
                        wsl = wqkT_sb[dc][:, wcol * 128:(wcol + 1) * 128]
                    for n in range(2):
                        nc.tensor.matmul(
                            ps[:, n * 512:(n + 1) * 512],
                            wsl,
                            xT_sb[dc][:, n * 512:(n + 1) * 512],
                            start=(dc == 0), stop=(dc == 7))
                qk = qp.tile([128, S], bf16, tag="qk", bufs=2,
                             name=f"qk{cc}_{which}")
                nc.vector.tensor_copy(qk[:], ps[:])
                # rotate_half partner copy.  The head-dim partition
                # order is [d0-15, d32-47, d16-31, d48-63] (host-side
                # permutation; scores are invariant to it), so partners
                # sit in 16-row halves of each 32-partition quadrant:
                # expressible as a DVE stream_shuffle (used for pair 0,
                # where the DMA queues are still draining weights) or as
                # 16-row SBUF strips.
                sw = qp.tile([128, S], bf16, tag="sw", bufs=2,
                             name=f"sw{cc}_{which}")
                if cc == 0:
                    nc.vector.stream_shuffle(
                        sw[:], qk[:],
                        list(range(16, 32)) + list(range(16)))
                else:
                    for quad in range(4):
                        for f in range(2):
                            o0 = quad * 32 + f * 16
                            i0 = quad * 32 + (1 - f) * 16
                            nc.sync.dma_start(sw[o0:o0 + 16, :],
                                              qk[i0:i0 + 16, :])
                for pss in range(2):
                    a = qp.tile([128, S], bf16, tag="ropeA", bufs=1,
                                name=f"ropeA{cc}_{which}_{pss}")
                    bb = qp.tile([128, S], bf16, tag="ropeB", bufs=1,
                                 name=f"ropeB{cc}_{which}_{pss}")
                    nc.vector.tensor_mul(a[:], qk[:], trigC_t[pss][:])
                    nc.vector.tensor_mul(bb[:], sw[:], trigS_t[pss][:])
                    nc.vector.tensor_add(roped[cc % 2][pss][which][:],
                                         a[:], bb[:])

            def v_proj(sc):
                """Project V chunk sc (128 seq rows) into vext[sc] with the
                2.0-const column 0 per head."""
                ps = bps.tile([128, S], f32, tag="big", bufs=3,
                              name=f"vps{sc}")
                for dc in range(8):
                    for n in range(2):
                        nc.tensor.matmul(
                            ps[:, n * 512:(n + 1) * 512],
                            xT_sb[dc][:, sc * 128:(sc + 1) * 128],
                            wvT_sb[dc][:, n * 512:(n + 1) * 512],
                            start=(dc == 0), stop=(dc == 7))
                vv = vext[sc][:].rearrange("p (h e) -> p h e", e=HD1)
                nc.vector.tensor_copy(
                    vv[:, :, 1:HD1],
                    ps[:].rearrange("p (h e) -> p h e", e=HD))
                nc.vector.memset(vv[:, :, 0:1], 2.0)

            # Phase = (cc, pss, n-half): scores+exp of a phase run while
            # the PREVIOUS phase's PV accumulates underneath (per-kc
            # interleave).  PV psum is then 2x [65,512] (2 banks) instead
            # of 4, buying a THIRD scores buffer: projection insertions
            # hold one buf while two still rotate, so the exp stream on
            # ScalarE never starves.
            quarters = {}
            nquart = {}

            def finish_phase(pd, pvt):
                cc, pss, n, _ = pd
                hE, hO = 2 * cc, 2 * cc + 1
                for g, h in ((0, hE), (1, hO)):
                    pvp = pvt[g]
                    pv_sb = sp.tile([HD1, 512], bf16, tag="pvsb", bufs=4,
                                    name=f"pvsb{pss}_{h}_{n}")
                    nc.vector.tensor_copy(pv_sb[:], pvp[:])
                    recf = sp.tile([1, 512], f32, tag="recf", bufs=2,
                                   name=f"recf{pss}_{h}_{n}")
                    nc.vector.reciprocal_approx_fast(recf[0:1, :],
                                                     pvp[0:1, :])
                    rec = sp.tile([1, 512], bf16, tag="rec", bufs=2,
                                  name=f"rec{pss}_{h}_{n}")
                    with nc.allow_low_precision(
                            reason="bf16 recip of softmax sums"):
                        nc.vector.tensor_copy(rec[0:1, :], recf[0:1, :])
                    bc_sb = sp.tile([HD1, 512], bf16, tag="bcsb", bufs=2,
                                    name=f"bcsb{pss}_{h}_{n}")
                    nc.gpsimd.partition_broadcast(bc_sb[:, :], rec[0:1, :],
                                                  channels=HD1)
                    ct = sp.tile([HD1, 512], bf16, tag=f"ct{pss}{g}{n}",
                                 bufs=1, name=f"ct{pss}_{h}_{n}")
                    nc.vector.tensor_mul(ct[:], pv_sb[:], bc_sb[:])
                    quarters[(cc, pss, g, n)] = ct
                nquart[cc] = nquart.get(cc, 0) + 2
                if nquart[cc] == 8:
                    for g, h in ((0, hE), (1, hO)):
                        hh = (h % 2) * 64
                        ah = sp.tile([HD1, S], bf16, tag="ah", bufs=2,
                                     name=f"ah{h}")
                        for nn in range(2):
                            nc.vector.tensor_add(
                                ah[:, nn * 512:(nn + 1) * 512],
                                quarters.pop((cc, 0, g, nn))[:],
                                quarters.pop((cc, 1, g, nn))[:])
                        nc.sync.dma_start(attn_b[cc][hh:hh + 64, :],
                                          ah[1:HD1, :])

            def run_phase(cc, pss, n, pending, extras=None):
                q1 = roped[cc % 2][pss][0]
                k1 = roped[cc % 2][pss][1]
                pvt = None
                if pending is not None:
                    pcc = pending[0]
                    pvt = [pvp_pool.tile([HD1, 512], f32, tag="pv", bufs=2,
                                         name=f"pv{pcc}_{g}")
                           for g in range(2)]
                es_list = []
                for kc in range(8):
                    scp = bps.tile([128, S], f32, tag="big", bufs=3,
                                   name=f"scp{pss}_{cc}_{kc}_{n}")
                    for g, hh in ((0, 0), (1, 64)):
                        nc.tensor.matmul(
                            scp[:, g * 512:(g + 1) * 512],
                            k1[hh:hh + 64, kc * 128:(kc + 1) * 128],
                            q1[hh:hh + 64, n * 512:(n + 1) * 512],
                            start=True, stop=True)
                    es = ep.tile([128, S], bf16, tag="expS", bufs=12,
                                 name=f"es{pss}_{cc}_{kc}_{n}")
                    nc.scalar.activation(es[:], scp[:], EXP, scale=0.125)
                    if extras and kc in extras:
                        extras[kc]()
                    if pending is not None:
                        pcc, ppss, pn, pes = pending
                        for g, h in ((0, 2 * pcc), (1, 2 * pcc + 1)):
                            nc.tensor.matmul(
                                pvt[g][:, :],
                                vext[kc][:, h * HD1:(h + 1) * HD1],
                                pes[kc][:, g * 512:(g + 1) * 512],
                                start=(kc == 0), stop=(kc == 7))
                    es_list.append(es)
                if pending is not None:
                    finish_phase(pending, pvt)
                return (cc, pss, n, es_list)

            # ---------- fused pipeline ----------
            # head-pair 0 prologue: emit both chunks' pass-0 rope before
            # any pass-1 rope so the first scores aren't FIFO-delayed
            cc0_parts = []
            SHUF = list(range(16, 32)) + list(range(16))
            for which in range(2):
                ps = bps.tile([128, S], f32, tag="big", bufs=3,
                              name=f"qkps0_{which}")
                for dc in range(8):
                    wsl = wqk0_sb[:, dc * 256 + which * 128:
                                  dc * 256 + (which + 1) * 128]
                    for n in range(2):
                        nc.tensor.matmul(
                            ps[:, n * 512:(n + 1) * 512], wsl,
                            xT_sb[dc][:, n * 512:(n + 1) * 512],
                            start=(dc == 0), stop=(dc == 7))
                qk = qp.tile([128, S], bf16, tag="qk", bufs=2,
                             name=f"qk0_{which}")
                sw = qp.tile([128, S], bf16, tag="sw", bufs=2,
                             name=f"sw0_{which}")
                cc0_parts.append((ps, qk, sw))
            # pass-0 rope of column-half 0 first: the first 4 key-chunks'
            # scores only touch roped columns 0-511, so they can start as
            # soon as this half lands
            abt = {}
            for h in range(2):
                cs = slice(h * 512, (h + 1) * 512)
                for which in range(2):
                    ps, qk, sw = cc0_parts[which]
                    nc.vector.tensor_copy(qk[:, cs], ps[:, cs])
                    nc.vector.stream_shuffle(sw[:, cs], qk[:, cs], SHUF)
                    a = qp.tile([128, S], bf16, tag="ropeA", bufs=1,
                                name=f"ropeA0_{which}")
                    bb = qp.tile([128, S], bf16, tag="ropeB", bufs=1,
                                 name=f"ropeB0_{which}")
                    if which in abt:
                        a, bb = abt[which]
                    else:
                        abt[which] = (a, bb)
                    nc.vector.tensor_mul(a[:, cs], qk[:, cs],
                                         trigC_t[0][:, cs])
                    nc.vector.tensor_mul(bb[:, cs], sw[:, cs],
                                         trigS_t[0][:, cs])
                    nc.vector.tensor_add(roped[0][0][which][:, cs],
                                         a[:, cs], bb[:, cs])
            for pss in (1,):
                for which in range(2):
                    ps, qk, sw = cc0_parts[which]
                    a2, b2 = abt[which]
                    nc.vector.tensor_mul(a2[:], qk[:], trigC_t[pss][:])
                    nc.vector.tensor_mul(b2[:], sw[:], trigS_t[pss][:])
                    nc.vector.tensor_add(roped[0][pss][which][:],
                                         a2[:], b2[:])
            v_proj(0)
            v_proj(1)

            pending = None
            for cc in range(8):
                for pss in range(2):
                    for n in range(2):
                        ex = None
                        if cc == 0 and pss == 0:
                            ex = {kc: (lambda s=3 * n + kc: v_proj(s))
                                  for kc in range(3 if n == 0 else 3)}
                            ex = {kc: (lambda s=2 + 3 * n + kc: v_proj(s))
                                  for kc in range(3)}
                        elif cc < 7 and pss == 1 and n == 0:
                            ex = {0: (lambda c=cc: qk_proj_chunk(c + 1, 0)),
                                  4: (lambda c=cc: qk_proj_chunk(c + 1, 1))}
                        pending = run_phase(cc, pss, n, pending, extras=ex)
            # drain the last phase's PV
            pcc = pending[0]
            pvt = [pvp_pool.tile([HD1, 512], f32, tag="pv", bufs=2,
                                 name=f"pvD_{g}") for g in range(2)]
            for kc in range(8):
                pes = pending[3]
                for g, h in ((0, 2 * pcc), (1, 2 * pcc + 1)):
                    nc.tensor.matmul(
                        pvt[g][:, :], vext[kc][:, h * HD1:(h + 1) * HD1],
                        pes[kc][:, g * 512:(g + 1) * 512],
                        start=(kc == 0), stop=(kc == 7))
            finish_phase(pending, pvt)

            # ---------- output projection ----------
            # partial accumulation over heads 0-6 of the next sc chunk is
            # emitted before the cc=7 finisher of the current one, so the PE
            # works while the last head-pair's normalize chain completes.
            def oproj_partial(sc, op):
                for cc in range(7):
                    for n in range(2):
                        nc.tensor.matmul(
                            op[:, n * 512:(n + 1) * 512],
                            attn_b[cc][:, sc * 128:(sc + 1) * 128],
                            woT_sb[cc][:, n * 512:(n + 1) * 512],
                            start=(cc == 0), stop=False)

            def oproj_finish(sc, op):
                for n in range(2):
                    nc.tensor.matmul(
                        op[:, n * 512:(n + 1) * 512],
                        attn_b[7][:, sc * 128:(sc + 1) * 128],
                        woT_sb[7][:, n * 512:(n + 1) * 512],
                        start=False, stop=True)
                ob = sp.tile([128, DM], f32, tag="ob", bufs=2,
                             name=f"ob{sc}")
                nc.vector.tensor_copy(ob[:], op[:])
                eng = nc.sync if sc % 2 == 0 else nc.scalar
                eng.dma_start(out_d[sc * 128:(sc + 1) * 128, :], ob[:])

            ops = {}
            ops[0] = bps.tile([128, DM], f32, tag="big", bufs=3, name="op0")
            oproj_partial(0, ops[0])
            for sc in range(8):
                if sc + 1 < 8:
                    ops[sc + 1] = bps.tile([128, DM], f32, tag="big",
                                           bufs=3, name=f"op{sc + 1}")
                    oproj_partial(sc + 1, ops[sc + 1])
                oproj_finish(sc, ops.pop(sc))

    nc.compile()
    _CACHE[key] = nc
    return nc


def _prep_inputs(hidden_states, cos, sin, w_qkv, w_o):
    bf = ml_dtypes.bfloat16
    xT = np.ascontiguousarray(
        hidden_states.transpose(0, 2, 1)).astype(bf)          # [B, DM, S]
    wqkT = np.ascontiguousarray(w_qkv[:2 * DM].T).astype(bf)  # [DM, 2DM]
    wvT = np.ascontiguousarray(w_qkv[2 * DM:].T).astype(bf)   # [DM, DM]
    woT = np.ascontiguousarray(w_o.T).astype(bf)              # [DM, DM]

    idx = np.arange(S).reshape(32, 32).T.reshape(-1)
    perm64 = np.concatenate([np.arange(0, 16), np.arange(32, 48),
                             np.arange(16, 32), np.arange(48, 64)])
    # permute every 64-wide head block of the q/k weight columns
    wqk_pc = wqkT.reshape(DM, 32, 64)[:, :, perm64].reshape(DM, 2 * DM)
    wqkT = np.ascontiguousarray(wqk_pc)
    d = perm64[np.arange(128) % HD]
    sign = np.where(d < 32, -1.0, 1.0).astype(np.float32)
    trig = np.concatenate([
        cos[:, d].T, cos[idx][:, d].T,
        sin[:, d].T * sign[:, None], sin[idx][:, d].T * sign[:, None],
    ], axis=1).astype(bf)                                     # [128, 4S]
    wqk0 = np.ascontiguousarray(
        np.concatenate([wqkT[:, 0:128], wqkT[:, 1024:1152]], axis=1)
        .reshape(8, 128, 256).transpose(1, 0, 2).reshape(128, 2048))
    shared = {"wqkT": wqkT, "wqk0": wqk0, "wvT": wvT, "woT": woT,
              "trig": np.ascontiguousarray(trig)}
    return [{"xT": np.ascontiguousarray(xT[b]), **shared} for b in range(B)]


def _install_ntff_hook():
    import sys, types
    if "antenv.axon_hooks" in sys.modules:
        return
    try:
        from trn_agent_boot.trn_boot import _ntff_profile_via_ctypes
        hook = _ntff_profile_via_ctypes('/opt/axon/libaxon_pjrt.so')
    except Exception:
        hook = None
    mod = types.ModuleType("antenv.axon_hooks")
    mod.get_axon_ntff_profile_hook = lambda: hook
    mod.set_axon_ntff_profile_hook = lambda h: None
    sys.modules["antenv.axon_hooks"] = mod


def kernel(hidden_states, cos, sin, w_qkv, w_o, _trace=False, _tmpdir=None):
    from concourse import bass_utils
    if _trace:
        _install_ntff_hook()
    nc = _build()
    in_maps = _prep_inputs(np.asarray(hidden_states, np.float32),
                           np.asarray(cos, np.float32),
                           np.asarray(sin, np.float32),
                           np.asarray(w_qkv, np.float32),
                           np.asarray(w_o, np.float32))
    res = bass_utils.run_bass_kernel_spmd(
        nc, in_maps, core_ids=list(range(NC)),
        trace=_trace, tmpdir=_tmpdir)
    out = np.stack([np.asarray(res.results[b]["out"], np.float32)
                    for b in range(B)])
    kernel.last_exec_time_ns = res.exec_time_ns
    return out
